# revision 11
# baseline (speedup 1.0000x reference)
"""Causal multi-head self-attention with RoPE on 8 Trainium2 NeuronCores.

Sharding: data-parallel over batch (B=4 -> 2 cores per batch) x tensor-parallel
over heads (16 heads -> 8 per core). Each core computes q/k/v projections for
its 8 heads, RoPE, causal attention, and a partial o_proj; the host sums the
two partial o_proj outputs per batch.

Layout strategy on device (per core):
  - x^T [1024, 2048] and pre-transposed weight shards are DMA'd in (host does
    the pure-layout transposes; all FLOPs run on device).
  - Q^T/K^T are produced head-major ([dk, s]) so attention needs no on-chip
    transposes; scores are computed transposed ([s_k, s_q]) so the softmax
    denominator comes from a ones-column augmented V matmul.
  - RoPE uses the "evens-then-odds" dk permutation (folded into the weight
    row order on the host), turning interleaved rotation into contiguous
    32-row block rotation; cos/sin tables are built on device from
    token_positions with Cody-Waite range reduction.
  - exp(scores/8) runs on the scalar engine straight out of PSUM; the causal
    mask is an affine_select on the diagonal 128-col block (GPSIMD).
  - Q^T/K^T spill to DRAM between the projection and attention phases to fit
    SBUF; V' and heads^T stay resident.
"""

import sys

sys.path.insert(0, "/opt/trn_rl_repo")

import numpy as np

import concourse.bass as bass
import concourse.tile as tile
from concourse import bacc, mybir
from concourse.bass_utils import run_bass_kernel_spmd
from concourse.masks import make_identity
from bass_rust import add_dep_helper

B, S, D, H = 4, 2048, 1024, 16
DK = D // H            # 64
HPC = H // 2           # 8 heads per core
DPC = HPC * DK         # 512 head dims per core
N_CORES = 8
HALF = DK // 2         # 32 rotary pairs
THETA = 10000.0

AF = mybir.ActivationFunctionType
F32 = mybir.dt.float32
F32R = mybir.dt.float32r
I32 = mybir.dt.int32

# Matmul input dtype for the big GEMMs: float32 = exact (4 cyc/row),
# float32r = TF32 (1 cyc/row at N>=256).
MM_DT = F32

TWO_PI = 2.0 * np.pi
# 3-term Cody-Waite split of 2*pi (c1/c2 have short mantissas so k*c is exact)
_CW_C1 = 6.28125
_CW_C2 = float(np.float32(9.67025756835937500e-4))
_CW_C3 = float(TWO_PI - _CW_C1 - np.float32(9.67025756835937500e-4))


def _build_program(debug=False):
    nc = bacc.Bacc("TRN2", target_bir_lowering=False, debug=False)

    xT = nc.dram_tensor("xT", [D, S], F32, kind="ExternalInput").ap()
    wqT = nc.dram_tensor("wqT", [D, DPC], F32, kind="ExternalInput").ap()
    wkT = nc.dram_tensor("wkT", [D, DPC], F32, kind="ExternalInput").ap()
    wvT = nc.dram_tensor("wvT", [D, DPC], F32, kind="ExternalInput").ap()
    woT = nc.dram_tensor("woT", [DPC, D], F32, kind="ExternalInput").ap()
    pos = nc.dram_tensor("pos", [S], I32, kind="ExternalInput").ap()
    invf_in = nc.dram_tensor("invf", [HALF], F32, kind="ExternalInput").ap()
    y = nc.dram_tensor("y", [S, D], F32, kind="ExternalOutput").ap()

    # Internal DRAM scratch for the Q^T/K^T spill: [q/k][e-chunk][128][S]
    qk_kind = "ExternalOutput" if debug else "Internal"
    qk_scr = nc.dram_tensor("qk_scr", [2, 4, 128, S], F32, kind=qk_kind).ap()
    dbg = None
    if debug:
        dbg = {
            "cs_dump": nc.dram_tensor("cs_dump", [2, 128, S], F32, kind="ExternalOutput").ap(),
            "vp_dump": nc.dram_tensor("vp_dump", [128, S // 128, HPC * (DK + 1)], F32, kind="ExternalOutput").ap(),
            "heads_dump": nc.dram_tensor("heads_dump", [128, DPC // 128, S], F32, kind="ExternalOutput").ap(),
            "e_dump": nc.dram_tensor("e_dump", [16, 4, 128, 512], F32, kind="ExternalOutput").ap(),
            "o_dump": nc.dram_tensor("o_dump", [4, 65, 512], F32, kind="ExternalOutput").ap(),
        }

    with tile.TileContext(nc) as tc:
        _emit(nc, tc, xT, wqT, wkT, wvT, woT, pos, invf_in, y, qk_scr, dbg)

    nc.compile()
    return nc


def _emit(nc, tc, xT, wqT, wkT, wvT, woT, pos, invf_in, y, qk_scr, dbg=None):
    import contextlib

    ctx = contextlib.ExitStack()
    with ctx:
        persist = ctx.enter_context(tc.tile_pool(name="persist", bufs=1))

        # ---------------- Phase B: rotary tables ----------------
        # Cbig/Sbig [128, S]: 32-row blocks [cos; cos; cos; cos] and
        # [-sin; sin; -sin; sin] so RoPE on a [128, s] slice of Q^T/K^T is
        #   q' = q * Cbig + (P_swap @ q) * Sbig
        cbig = persist.tile([128, S], F32)
        sbig = persist.tile([128, S], F32)
        p_swap = persist.tile([128, 128], F32)
        identity = persist.tile([128, 128], F32)
        make_identity(nc, identity)
        # P_swap: swap 32-row blocks within each 64-block (rows 0<->32, 64<->96)
        nc.gpsimd.memset(p_swap, 0.0)
        for blk in range(4):
            src = (blk ^ 1) * 32
            nc.sync.dma_start(
                out=p_swap[blk * 32:(blk + 1) * 32, :],
                in_=identity[src:src + 32, :],
            )

        with tc.tile_pool(name="tables", bufs=1) as tbl, \
             tc.tile_pool(name="tbl_ps", bufs=1, space="PSUM") as tbl_ps:
            posi = tbl.tile([1, S], I32)
            nc.sync.dma_start(out=posi, in_=pos.unsqueeze(0))
            posf = tbl.tile([1, S], F32)
            nc.vector.tensor_copy(posf, posi)

            invf = tbl.tile([1, HALF], F32)
            nc.sync.dma_start(out=invf, in_=invf_in.unsqueeze(0))

            ang_ps = tbl_ps.tile([HALF, S], F32)
            for j in range(S // 512):
                nc.tensor.matmul(ang_ps[:, j * 512:(j + 1) * 512], invf,
                                 posf[:, j * 512:(j + 1) * 512],
                                 start=True, stop=True)
            ang = tbl.tile([HALF, S], F32)
            nc.scalar.copy(ang, ang_ps)

            k_i = tbl.tile([HALF, S], I32)
            nc.scalar.activation(k_i, ang, AF.Copy, scale=float(1.0 / TWO_PI))
            k_f = tbl.tile([HALF, S], F32)
            nc.vector.tensor_copy(k_f, k_i)
            ang_red = tbl.tile([HALF, S], F32)
            nc.vector.cody_waite_cascade(ang_red, ang, k_f, _CW_C1, _CW_C2, _CW_C3)

            sin_arg = tbl.tile([HALF, S], F32)
            cos_arg = tbl.tile([HALF, S], F32)
            nc.vector.add_range_wrap(sin_arg, ang_red, 0.0, float(np.pi), TWO_PI)
            nc.vector.add_range_wrap(cos_arg, ang_red, float(np.pi / 2), float(np.pi), TWO_PI)

            # cos into cbig rows 0:32; sin into sbig rows 32:64 (+), 0:32 (-)
            nc.scalar.activation(cbig[0:HALF, :], cos_arg, AF.Sin)
            s_pos = tbl.tile([HALF, S], F32)
            nc.scalar.activation(s_pos, sin_arg, AF.Sin)
            nc.scalar.mul(sbig[0:HALF, :], s_pos, -1.0)
            nc.sync.dma_start(out=sbig[HALF:2 * HALF, :], in_=s_pos)
            # replicate 64-row block to rows 64:128
            nc.sync.dma_start(out=cbig[HALF:2 * HALF, :], in_=cbig[0:HALF, :])
            nc.sync.dma_start(out=cbig[64:128, :], in_=cbig[0:64, :])
            nc.sync.dma_start(out=sbig[64:128, :], in_=sbig[0:64, :])
            if dbg is not None:
                nc.sync.dma_start(out=dbg["cs_dump"][0], in_=cbig)
                nc.sync.dma_start(out=dbg["cs_dump"][1], in_=sbig)

        # ---------------- Phase C: projections + RoPE + spill ----------------
        xt_pool = tc.alloc_tile_pool(name="xt", bufs=1)
        xt_t = xt_pool.tile([128, D // 128, S], MM_DT)
        nc.sync.dma_start(out=xt_t, in_=xT.bitcast(MM_DT).rearrange("(c p) s -> p c s", p=128))

        vp = persist.tile([128, S // 128, HPC * (DK + 1)], MM_DT)
        # ones column of V' (device col DK of each head block)
        nc.vector.memset(
            vp.rearrange("p s (h c) -> p s h c", h=HPC)[:, :, :, DK:DK + 1], 1.0)

        spill_insts = []
        for qk_idx, w_dram in ((0, wqT), (1, wkT)):
            with tc.tile_pool(name=f"w{qk_idx}", bufs=1) as wpool, \
                 tc.tile_pool(name=f"proj{qk_idx}", bufs=4) as proj_sb, \
                 tc.tile_pool(name=f"rope{qk_idx}", bufs=4) as rope_sb, \
                 tc.tile_pool(name=f"ps{qk_idx}", bufs=4, space="PSUM") as proj_ps, \
                 tc.tile_pool(name=f"psw{qk_idx}", bufs=2, space="PSUM") as swap_ps:
                w_t = wpool.tile([128, D // 128, DPC], MM_DT, name=f"w_t{qk_idx}")
                nc.sync.dma_start(
                    out=w_t, in_=w_dram.bitcast(MM_DT).rearrange("(c p) e -> p c e", p=128))
                for et in range(DPC // 128):
                    for sc in range(S // 512):
                        ssl = bass.ts(sc, 512)
                        p_t = proj_ps.tile([128, 512], F32, name="p_t")
                        for dc in range(D // 128):
                            nc.tensor.matmul(
                                p_t, w_t[:, dc, et * 128:(et + 1) * 128],
                                xt_t[:, dc, ssl],
                                start=(dc == 0), stop=(dc == D // 128 - 1))
                        qt_sb = proj_sb.tile([128, 512], F32, name="qt_sb")
                        nc.scalar.copy(qt_sb, p_t)
                        # RoPE: out = qt*C + (P_swap @ qt)*S
                        sw_ps = swap_ps.tile([128, 512], F32, name="sw_ps")
                        nc.tensor.matmul(sw_ps, p_swap, qt_sb, start=True, stop=True)
                        g1 = rope_sb.tile([128, 512], F32, name="g1")
                        nc.gpsimd.tensor_mul(g1, qt_sb, cbig[:, ssl])
                        d1 = rope_sb.tile([128, 512], F32, name="d1")
                        nc.vector.tensor_mul(d1, sw_ps, sbig[:, ssl])
                        qtr = rope_sb.tile([128, 512], F32, name="qtr")
                        nc.vector.tensor_add(qtr, g1, d1)
                        spill = nc.sync.dma_start(out=qk_scr[qk_idx, et, :, ssl], in_=qtr)
                        spill_insts.append(spill.ins)

        with tc.tile_pool(name="wv", bufs=1) as wvpool, \
             tc.tile_pool(name="psv", bufs=4, space="PSUM") as v_ps:
            wv_t = wvpool.tile([128, D // 128, DPC], MM_DT)
            nc.sync.dma_start(
                out=wv_t, in_=wvT.bitcast(MM_DT).rearrange("(c p) e -> p c e", p=128))
            vp_heads = vp.rearrange("p s (h c) -> p s h c", h=HPC)
            for st in range(S // 128):
                pv_t = v_ps.tile([128, 512], F32, name="pv_t")
                for dc in range(D // 128):
                    nc.tensor.matmul(
                        pv_t, xt_t[:, dc, st * 128:(st + 1) * 128], wv_t[:, dc, :],
                        start=(dc == 0), stop=(dc == D // 128 - 1))
                nc.scalar.copy(vp_heads[:, st, :, 0:DK], pv_t)

        if dbg is not None:
            nc.sync.dma_start(out=dbg["vp_dump"], in_=vp.bitcast(F32))
        xt_pool.release()

        # ---------------- Phase D: attention ----------------
        heads_pool = ctx.enter_context(tc.tile_pool(name="heads", bufs=1))
        heads_t = heads_pool.tile([128, DPC // 128, S], MM_DT)
        with tc.tile_pool(name="qk_sb", bufs=2) as qk_sb, \
             tc.tile_pool(name="expp", bufs=6) as exp_pool, \
             tc.tile_pool(name="norm", bufs=4) as norm_pool, \
             tc.tile_pool(name="ps_s", bufs=4, space="PSUM") as s_ps, \
             tc.tile_pool(name="ps_o", bufs=2, space="PSUM") as o_ps:
            for h in range(HPC):
                ec, ro = h // 2, (h % 2) * DK
                qt_h = qk_sb.tile([DK, S], MM_DT, name="qt_h")
                ld_q = nc.sync.dma_start(out=qt_h, in_=qk_scr.bitcast(MM_DT)[0, ec, ro:ro + DK, :])
                kt_h = qk_sb.tile([DK, S], MM_DT, name="kt_h")
                ld_k = nc.sync.dma_start(out=kt_h, in_=qk_scr.bitcast(MM_DT)[1, ec, ro:ro + DK, :])
                for sp in spill_insts:
                    add_dep_helper(ld_q.ins, sp, True, "qk spill->load")
                    add_dep_helper(ld_k.ins, sp, True, "qk spill->load")
                for qc in range(S // 512):
                    o_t = o_ps.tile([DK + 1, 512], F32, name="o_t")
                    n_kt = 4 * qc + 4
                    for kt in range(n_kt):
                        diag = (kt // 4 == qc)
                        # diagonal-chunk strips only cover q >= 128*kt; the
                        # first 128 cols of that range get the triangle mask
                        co = 128 * (kt % 4) if diag else 0
                        n = 512 - co
                        e_t = exp_pool.tile([128, 512], MM_DT, name="e_t")
                        sc_t = s_ps.tile([128, 512], F32, name="sc_t")
                        nc.tensor.matmul(
                            sc_t[:, 0:n], kt_h[:, kt * 128:(kt + 1) * 128],
                            qt_h[:, qc * 512 + co:(qc + 1) * 512],
                            start=True, stop=True)
                        nc.scalar.activation(e_t[:, 0:n], sc_t[:, 0:n], AF.Exp,
                                             scale=float(1.0 / np.sqrt(DK)))
                        if diag:
                            nc.gpsimd.affine_select(
                                out=e_t[:, 0:128], in_=e_t[:, 0:128],
                                pattern=[[1, 128]], base=0, channel_multiplier=-1,
                                compare_op=mybir.AluOpType.is_ge, fill=0.0)
                        nc.tensor.matmul(
                            o_t[:, co:512],
                            vp[:, kt, h * (DK + 1):(h + 1) * (DK + 1)],
                            e_t[:, 0:n],
                            start=(kt == 0), stop=(kt == n_kt - 1))
                        if dbg is not None and h == 0:
                            nc.sync.dma_start(
                                out=dbg["e_dump"][kt, qc, :, 0:n], in_=e_t.bitcast(F32)[:, 0:n])
                    if dbg is not None and h == 0:
                        ob = norm_pool.tile([65, 512], F32, name="ob")
                        nc.scalar.copy(ob, o_t)
                        nc.sync.dma_start(out=dbg["o_dump"][qc], in_=ob)
                    dsb = norm_pool.tile([1, 512], F32, name="dsb")
                    nc.scalar.copy(dsb, o_t[DK:DK + 1, :])
                    recip = norm_pool.tile([1, 512], F32, name="recip")
                    nc.vector.reciprocal_approx_fast(recip, dsb)
                    rb = norm_pool.tile([DK, 512], F32, name="rb")
                    nc.gpsimd.partition_broadcast(rb, recip)
                    hn = norm_pool.tile([DK, 512], F32, name="hn")
                    nc.vector.tensor_mul(hn, o_t[0:DK, :], rb)
                    nc.sync.dma_start(
                        out=heads_t[ro:ro + DK, ec, bass.ts(qc, 512)], in_=hn)

        if dbg is not None:
            nc.sync.dma_start(out=dbg["heads_dump"], in_=heads_t.bitcast(F32))

        # ---------------- Phase E: o_proj ----------------
        with tc.tile_pool(name="wo", bufs=1) as wopool, \
             tc.tile_pool(name="yout", bufs=4) as ypool, \
             tc.tile_pool(name="ps_y", bufs=4, space="PSUM") as y_ps:
            wo_t = wopool.tile([128, DPC // 128, D], MM_DT)
            nc.sync.dma_start(
                out=wo_t, in_=woT.bitcast(MM_DT).rearrange("(c p) e -> p c e", p=128))
            for st in range(S // 128):
                for nb in range(D // 512):
                    py_t = y_ps.tile([128, 512], F32, name="py_t")
                    for dc in range(DPC // 128):
                        nc.tensor.matmul(
                            py_t, heads_t[:, dc, st * 128:(st + 1) * 128],
                            wo_t[:, dc, bass.ts(nb, 512)],
                            start=(dc == 0), stop=(dc == DPC // 128 - 1))
                    y_sb = ypool.tile([128, 512], F32, name="y_sb")
                    nc.vector.tensor_copy(y_sb, py_t)
                    nc.sync.dma_start(
                        out=y[st * 128:(st + 1) * 128, bass.ts(nb, 512)], in_=y_sb)


def _host_inv_freq():
    import jax
    import jax.numpy as jnp
    with jax.default_device(jax.devices("cpu")[0]):
        v = 1.0 / (THETA ** (jnp.arange(HALF, dtype=jnp.float32) * 2.0 / DK))
        return np.asarray(jax.device_get(v)).astype(np.float32)


_program_cache = None


def _get_program():
    global _program_cache
    if _program_cache is None:
        _program_cache = _build_program()
    return _program_cache


# dk permutation: evens then odds within each head's 64 dims
_PERM64 = np.concatenate([np.arange(0, DK, 2), np.arange(1, DK, 2)])


def kernel(x, Wq, Wk, Wv, Wo, token_positions):
    x = np.asarray(x, dtype=np.float32)
    Wq = np.asarray(Wq, dtype=np.float32)
    Wk = np.asarray(Wk, dtype=np.float32)
    Wv = np.asarray(Wv, dtype=np.float32)
    Wo = np.asarray(Wo, dtype=np.float32)
    pos_np = np.ascontiguousarray(np.asarray(token_positions, dtype=np.int32))
    invf_np = _host_inv_freq()

    nc = _get_program()

    in_maps = []
    for c in range(N_CORES):
        b, hg = c // 2, c % 2
        rows = hg * DPC + np.concatenate(
            [h * DK + _PERM64 for h in range(HPC)])
        in_maps.append({
            "xT": np.ascontiguousarray(x[b].T),
            "wqT": np.ascontiguousarray(Wq[rows, :].T),
            "wkT": np.ascontiguousarray(Wk[rows, :].T),
            "wvT": np.ascontiguousarray(Wv[hg * DPC:(hg + 1) * DPC, :].T),
            "woT": np.ascontiguousarray(Wo[:, hg * DPC:(hg + 1) * DPC].T),
            "pos": pos_np,
            "invf": invf_np,
        })

    res = run_bass_kernel_spmd(nc, in_maps, list(range(N_CORES)))
    out = np.empty((B, S, D), dtype=np.float32)
    for b in range(B):
        out[b] = res.results[2 * b]["y"] + res.results[2 * b + 1]["y"]
    return out


# revision 14
# speedup vs baseline: 1.7664x; 1.7664x over previous
"""Causal multi-head self-attention with RoPE on 8 Trainium2 NeuronCores.

Sharding: data-parallel over batch (B=4 -> 2 cores per batch) x tensor-parallel
over heads (16 heads -> 8 per core). Each core computes q/k/v projections for
its 8 heads, RoPE, causal attention, and a partial o_proj; the host sums the
two partial o_proj outputs per batch.

Layout strategy on device (per core):
  - x^T [1024, 2048] and pre-transposed weight shards are DMA'd in (host does
    the pure-layout transposes; all FLOPs run on device).
  - Q^T/K^T are produced head-major ([dk, s]) so attention needs no on-chip
    transposes; scores are computed transposed ([s_k, s_q]) so the softmax
    denominator comes from a ones-column augmented V matmul.
  - RoPE uses the "evens-then-odds" dk permutation (folded into the weight
    row order on the host), turning interleaved rotation into contiguous
    32-row block rotation; cos/sin tables are built on device from
    token_positions with Cody-Waite range reduction.
  - exp(scores/8) runs on the scalar engine straight out of PSUM; the causal
    mask is an affine_select on the diagonal 128-col block (GPSIMD).
  - Q^T/K^T spill to DRAM between the projection and attention phases to fit
    SBUF; V' and heads^T stay resident.
"""

import sys

sys.path.insert(0, "/opt/trn_rl_repo")

import numpy as np

import concourse.bass as bass
import concourse.tile as tile
from concourse import bacc, mybir
from concourse.bass_utils import run_bass_kernel_spmd
from concourse.masks import make_identity
from bass_rust import add_dep_helper

B, S, D, H = 4, 2048, 1024, 16
DK = D // H            # 64
HPC = H // 2           # 8 heads per core
DPC = HPC * DK         # 512 head dims per core
N_CORES = 8
HALF = DK // 2         # 32 rotary pairs
THETA = 10000.0

AF = mybir.ActivationFunctionType
F32 = mybir.dt.float32
F32R = mybir.dt.float32r
I32 = mybir.dt.int32

# Matmul input dtype for the big GEMMs: float32 = exact (4 cyc/row),
# float32r = TF32 (1 cyc/row at N>=256).
import os as _os
MM_DT = F32R if _os.environ.get("MMDT", "f32") == "f32r" else F32

TWO_PI = 2.0 * np.pi
# 3-term Cody-Waite split of 2*pi (c1/c2 have short mantissas so k*c is exact)
_CW_C1 = 6.28125
_CW_C2 = float(np.float32(9.67025756835937500e-4))
_CW_C3 = float(TWO_PI - _CW_C1 - np.float32(9.67025756835937500e-4))


def _build_program(debug=False):
    nc = bacc.Bacc("TRN2", target_bir_lowering=False, debug=False)

    xT = nc.dram_tensor("xT", [D, S], F32, kind="ExternalInput").ap()
    wqT = nc.dram_tensor("wqT", [D, DPC], F32, kind="ExternalInput").ap()
    wkT = nc.dram_tensor("wkT", [D, DPC], F32, kind="ExternalInput").ap()
    wvT = nc.dram_tensor("wvT", [D, DPC], F32, kind="ExternalInput").ap()
    woT = nc.dram_tensor("woT", [DPC, D], F32, kind="ExternalInput").ap()
    pos = nc.dram_tensor("pos", [S], I32, kind="ExternalInput").ap()
    invf_in = nc.dram_tensor("invf", [HALF], F32, kind="ExternalInput").ap()
    y = nc.dram_tensor("y", [S, D], F32, kind="ExternalOutput").ap()

    # Internal DRAM scratch for the Q^T/K^T spill: [q/k][e-chunk][128][S]
    qk_kind = "ExternalOutput" if debug else "Internal"
    qk_scr = nc.dram_tensor("qk_scr", [2, 4, 128, S], F32, kind=qk_kind).ap()
    dbg = None
    if debug:
        dbg = {
            "cs_dump": nc.dram_tensor("cs_dump", [2, 128, S], F32, kind="ExternalOutput").ap(),
            "vp_dump": nc.dram_tensor("vp_dump", [128, S // 128, HPC * (DK + 1)], F32, kind="ExternalOutput").ap(),
            "heads_dump": nc.dram_tensor("heads_dump", [128, DPC // 128, S], F32, kind="ExternalOutput").ap(),
            "e_dump": nc.dram_tensor("e_dump", [16, 4, 128, 512], F32, kind="ExternalOutput").ap(),
            "o_dump": nc.dram_tensor("o_dump", [4, 65, 512], F32, kind="ExternalOutput").ap(),
        }

    with tile.TileContext(nc) as tc:
        _emit(nc, tc, xT, wqT, wkT, wvT, woT, pos, invf_in, y, qk_scr, dbg)

    nc.compile()
    return nc


def _emit(nc, tc, xT, wqT, wkT, wvT, woT, pos, invf_in, y, qk_scr, dbg=None):
    import contextlib

    ctx = contextlib.ExitStack()
    with ctx:
        persist = ctx.enter_context(tc.tile_pool(name="persist", bufs=1))

        # ---------------- Phase B: rotary tables ----------------
        # Cbig/Sbig [128, S]: 32-row blocks [cos; cos; cos; cos] and
        # [-sin; sin; -sin; sin] so RoPE on a [128, s] slice of Q^T/K^T is
        #   q' = q * Cbig + (P_swap @ q) * Sbig
        cbig = persist.tile([128, S], F32)
        sbig = persist.tile([128, S], F32)
        p_swap = persist.tile([128, 128], F32)
        identity = persist.tile([128, 128], F32)
        make_identity(nc, identity)
        # P_swap: swap 32-row blocks within each 64-block (rows 0<->32, 64<->96)
        nc.gpsimd.memset(p_swap, 0.0)
        for blk in range(4):
            src = (blk ^ 1) * 32
            nc.sync.dma_start(
                out=p_swap[blk * 32:(blk + 1) * 32, :],
                in_=identity[src:src + 32, :],
            )

        with tc.tile_pool(name="tables", bufs=1) as tbl, \
             tc.tile_pool(name="tbl_ps", bufs=1, space="PSUM") as tbl_ps:
            posi = tbl.tile([1, S], I32)
            nc.sync.dma_start(out=posi, in_=pos.unsqueeze(0))
            posf = tbl.tile([1, S], F32)
            nc.vector.tensor_copy(posf, posi)

            invf = tbl.tile([1, HALF], F32)
            nc.sync.dma_start(out=invf, in_=invf_in.unsqueeze(0))

            ang_ps = tbl_ps.tile([HALF, S], F32)
            for j in range(S // 512):
                nc.tensor.matmul(ang_ps[:, j * 512:(j + 1) * 512], invf,
                                 posf[:, j * 512:(j + 1) * 512],
                                 start=True, stop=True)
            ang = tbl.tile([HALF, S], F32)
            nc.scalar.copy(ang, ang_ps)

            k_i = tbl.tile([HALF, S], I32)
            nc.scalar.activation(k_i, ang, AF.Copy, scale=float(1.0 / TWO_PI))
            k_f = tbl.tile([HALF, S], F32)
            nc.vector.tensor_copy(k_f, k_i)
            ang_red = tbl.tile([HALF, S], F32)
            nc.vector.cody_waite_cascade(ang_red, ang, k_f, _CW_C1, _CW_C2, _CW_C3)

            sin_arg = tbl.tile([HALF, S], F32)
            cos_arg = tbl.tile([HALF, S], F32)
            nc.vector.add_range_wrap(sin_arg, ang_red, 0.0, float(np.pi), TWO_PI)
            nc.vector.add_range_wrap(cos_arg, ang_red, float(np.pi / 2), float(np.pi), TWO_PI)

            # cos into cbig rows 0:32; sin into sbig rows 32:64 (+), 0:32 (-)
            nc.scalar.activation(cbig[0:HALF, :], cos_arg, AF.Sin)
            s_pos = tbl.tile([HALF, S], F32)
            nc.scalar.activation(s_pos, sin_arg, AF.Sin)
            nc.scalar.mul(sbig[0:HALF, :], s_pos, -1.0)
            nc.sync.dma_start(out=sbig[HALF:2 * HALF, :], in_=s_pos)
            # replicate 64-row block to rows 64:128
            nc.sync.dma_start(out=cbig[HALF:2 * HALF, :], in_=cbig[0:HALF, :])
            nc.sync.dma_start(out=cbig[64:128, :], in_=cbig[0:64, :])
            nc.sync.dma_start(out=sbig[64:128, :], in_=sbig[0:64, :])
            if dbg is not None:
                nc.sync.dma_start(out=dbg["cs_dump"][0], in_=cbig)
                nc.sync.dma_start(out=dbg["cs_dump"][1], in_=sbig)

        # ---------------- Phase C: projections + RoPE + spill ----------------
        xt_pool = tc.alloc_tile_pool(name="xt", bufs=1)
        xt_t = xt_pool.tile([128, D // 128, S], MM_DT)
        nc.sync.dma_start(out=xt_t, in_=xT.bitcast(MM_DT).rearrange("(c p) s -> p c s", p=128))

        vp = persist.tile([128, S // 128, HPC * (DK + 1)], MM_DT)
        # ones column of V' (device col DK of each head block); memset can't
        # write f32r, so memset an f32 column and broadcast-copy via ScalarE
        ones_col = persist.tile([128, 1], F32)
        nc.vector.memset(ones_col, 1.0)
        nc.scalar.copy(
            vp.rearrange("p s (h c) -> p s h c", h=HPC)[:, :, :, DK:DK + 1],
            ones_col.to_broadcast((128, S // 128, HPC, 1)))

        spill_insts = []
        for qk_idx, w_dram in ((0, wqT), (1, wkT)):
            with tc.tile_pool(name=f"w{qk_idx}", bufs=1) as wpool, \
                 tc.tile_pool(name=f"proj{qk_idx}", bufs=4) as proj_sb, \
                 tc.tile_pool(name=f"rope{qk_idx}", bufs=4) as rope_sb, \
                 tc.tile_pool(name=f"ps{qk_idx}", bufs=4, space="PSUM") as proj_ps, \
                 tc.tile_pool(name=f"psw{qk_idx}", bufs=2, space="PSUM") as swap_ps:
                w_t = wpool.tile([128, D // 128, DPC], MM_DT, name=f"w_t{qk_idx}")
                nc.sync.dma_start(
                    out=w_t, in_=w_dram.bitcast(MM_DT).rearrange("(c p) e -> p c e", p=128))
                for et in range(DPC // 128):
                    for sc in range(S // 512):
                        ssl = bass.ts(sc, 512)
                        p_t = proj_ps.tile([128, 512], F32, name="p_t")
                        for dc in range(D // 128):
                            nc.tensor.matmul(
                                p_t, w_t[:, dc, et * 128:(et + 1) * 128],
                                xt_t[:, dc, ssl],
                                start=(dc == 0), stop=(dc == D // 128 - 1))
                        qt_sb = proj_sb.tile([128, 512], F32, name="qt_sb")
                        nc.scalar.copy(qt_sb, p_t)
                        # RoPE: out = qt*C + (P_swap @ qt)*S
                        sw_ps = swap_ps.tile([128, 512], F32, name="sw_ps")
                        nc.tensor.matmul(sw_ps, p_swap, qt_sb, start=True, stop=True)
                        g1 = rope_sb.tile([128, 512], F32, name="g1")
                        nc.gpsimd.tensor_mul(g1, qt_sb, cbig[:, ssl])
                        d1 = rope_sb.tile([128, 512], F32, name="d1")
                        nc.vector.tensor_mul(d1, sw_ps, sbig[:, ssl])
                        qtr = rope_sb.tile([128, 512], F32, name="qtr")
                        nc.vector.tensor_add(qtr, g1, d1)
                        spill = nc.sync.dma_start(out=qk_scr[qk_idx, et, :, ssl], in_=qtr)
                        spill_insts.append(spill.ins)

        with tc.tile_pool(name="wv", bufs=1) as wvpool, \
             tc.tile_pool(name="psv", bufs=4, space="PSUM") as v_ps:
            wv_t = wvpool.tile([128, D // 128, DPC], MM_DT)
            nc.sync.dma_start(
                out=wv_t, in_=wvT.bitcast(MM_DT).rearrange("(c p) e -> p c e", p=128))
            vp_heads = vp.rearrange("p s (h c) -> p s h c", h=HPC)
            for st in range(S // 128):
                pv_t = v_ps.tile([128, 512], F32, name="pv_t")
                for dc in range(D // 128):
                    nc.tensor.matmul(
                        pv_t, xt_t[:, dc, st * 128:(st + 1) * 128], wv_t[:, dc, :],
                        start=(dc == 0), stop=(dc == D // 128 - 1))
                nc.scalar.copy(vp_heads[:, st, :, 0:DK], pv_t)

        if dbg is not None:
            nc.sync.dma_start(out=dbg["vp_dump"], in_=vp.bitcast(F32))
        xt_pool.release()

        # ---------------- Phase D: attention ----------------
        heads_pool = ctx.enter_context(tc.tile_pool(name="heads", bufs=1))
        heads_t = heads_pool.tile([128, DPC // 128, S], MM_DT)
        with tc.tile_pool(name="qk_sb", bufs=2) as qk_sb, \
             tc.tile_pool(name="expp", bufs=6) as exp_pool, \
             tc.tile_pool(name="norm", bufs=4) as norm_pool, \
             tc.tile_pool(name="ps_s", bufs=4, space="PSUM") as s_ps, \
             tc.tile_pool(name="ps_o", bufs=2, space="PSUM") as o_ps:
            for h in range(HPC):
                ec, ro = h // 2, (h % 2) * DK
                qt_h = qk_sb.tile([DK, S], MM_DT, name="qt_h")
                ld_q = nc.sync.dma_start(out=qt_h, in_=qk_scr.bitcast(MM_DT)[0, ec, ro:ro + DK, :])
                kt_h = qk_sb.tile([DK, S], MM_DT, name="kt_h")
                ld_k = nc.sync.dma_start(out=kt_h, in_=qk_scr.bitcast(MM_DT)[1, ec, ro:ro + DK, :])
                for sp in spill_insts:
                    add_dep_helper(ld_q.ins, sp, True, "qk spill->load")
                    add_dep_helper(ld_k.ins, sp, True, "qk spill->load")
                for qc in range(S // 512):
                    o_t = o_ps.tile([DK + 1, 512], F32, name="o_t")
                    n_kt = 4 * qc + 4
                    for kt in range(n_kt):
                        diag = (kt // 4 == qc)
                        # diagonal-chunk strips only cover q >= 128*kt; the
                        # first 128 cols of that range get the triangle mask
                        co = 128 * (kt % 4) if diag else 0
                        n = 512 - co
                        e_t = exp_pool.tile([128, 512], MM_DT, name="e_t")
                        sc_t = s_ps.tile([128, 512], F32, name="sc_t")
                        nc.tensor.matmul(
                            sc_t[:, 0:n], kt_h[:, kt * 128:(kt + 1) * 128],
                            qt_h[:, qc * 512 + co:(qc + 1) * 512],
                            start=True, stop=True)
                        nc.scalar.activation(e_t[:, 0:n], sc_t[:, 0:n], AF.Exp,
                                             scale=float(1.0 / np.sqrt(DK)))
                        if diag:
                            nc.gpsimd.affine_select(
                                out=e_t[:, 0:128], in_=e_t[:, 0:128],
                                pattern=[[1, 128]], base=0, channel_multiplier=-1,
                                compare_op=mybir.AluOpType.is_ge, fill=0.0)
                        nc.tensor.matmul(
                            o_t[:, co:512],
                            vp[:, kt, h * (DK + 1):(h + 1) * (DK + 1)],
                            e_t[:, 0:n],
                            start=(kt == 0), stop=(kt == n_kt - 1))
                        if dbg is not None and h == 0:
                            nc.sync.dma_start(
                                out=dbg["e_dump"][kt, qc, :, 0:n], in_=e_t.bitcast(F32)[:, 0:n])
                    if dbg is not None and h == 0:
                        ob = norm_pool.tile([65, 512], F32, name="ob")
                        nc.scalar.copy(ob, o_t)
                        nc.sync.dma_start(out=dbg["o_dump"][qc], in_=ob)
                    dsb = norm_pool.tile([1, 512], F32, name="dsb")
                    nc.scalar.copy(dsb, o_t[DK:DK + 1, :])
                    recip = norm_pool.tile([1, 512], F32, name="recip")
                    nc.vector.reciprocal_approx_fast(recip, dsb)
                    rb = norm_pool.tile([DK, 512], F32, name="rb")
                    nc.gpsimd.partition_broadcast(rb, recip)
                    hn = norm_pool.tile([DK, 512], F32, name="hn")
                    nc.vector.tensor_mul(hn, o_t[0:DK, :], rb)
                    nc.sync.dma_start(
                        out=heads_t[ro:ro + DK, ec, bass.ts(qc, 512)],
                        in_=hn.bitcast(MM_DT))

        if dbg is not None:
            nc.sync.dma_start(out=dbg["heads_dump"], in_=heads_t.bitcast(F32))

        # ---------------- Phase E: o_proj ----------------
        with tc.tile_pool(name="wo", bufs=1) as wopool, \
             tc.tile_pool(name="yout", bufs=4) as ypool, \
             tc.tile_pool(name="ps_y", bufs=4, space="PSUM") as y_ps:
            wo_t = wopool.tile([128, DPC // 128, D], MM_DT)
            nc.sync.dma_start(
                out=wo_t, in_=woT.bitcast(MM_DT).rearrange("(c p) e -> p c e", p=128))
            for st in range(S // 128):
                for nb in range(D // 512):
                    py_t = y_ps.tile([128, 512], F32, name="py_t")
                    for dc in range(DPC // 128):
                        nc.tensor.matmul(
                            py_t, heads_t[:, dc, st * 128:(st + 1) * 128],
                            wo_t[:, dc, bass.ts(nb, 512)],
                            start=(dc == 0), stop=(dc == DPC // 128 - 1))
                    y_sb = ypool.tile([128, 512], F32, name="y_sb")
                    nc.vector.tensor_copy(y_sb, py_t)
                    nc.sync.dma_start(
                        out=y[st * 128:(st + 1) * 128, bass.ts(nb, 512)], in_=y_sb)


def _host_inv_freq():
    import jax
    import jax.numpy as jnp
    with jax.default_device(jax.devices("cpu")[0]):
        v = 1.0 / (THETA ** (jnp.arange(HALF, dtype=jnp.float32) * 2.0 / DK))
        return np.asarray(jax.device_get(v)).astype(np.float32)


_program_cache = None


def _get_program():
    global _program_cache
    if _program_cache is None:
        _program_cache = _build_program()
    return _program_cache


# dk permutation: evens then odds within each head's 64 dims
_PERM64 = np.concatenate([np.arange(0, DK, 2), np.arange(1, DK, 2)])


def kernel(x, Wq, Wk, Wv, Wo, token_positions):
    x = np.asarray(x, dtype=np.float32)
    Wq = np.asarray(Wq, dtype=np.float32)
    Wk = np.asarray(Wk, dtype=np.float32)
    Wv = np.asarray(Wv, dtype=np.float32)
    Wo = np.asarray(Wo, dtype=np.float32)
    pos_np = np.ascontiguousarray(np.asarray(token_positions, dtype=np.int32))
    invf_np = _host_inv_freq()

    nc = _get_program()

    in_maps = []
    for c in range(N_CORES):
        b, hg = c // 2, c % 2
        rows = hg * DPC + np.concatenate(
            [h * DK + _PERM64 for h in range(HPC)])
        in_maps.append({
            "xT": np.ascontiguousarray(x[b].T),
            "wqT": np.ascontiguousarray(Wq[rows, :].T),
            "wkT": np.ascontiguousarray(Wk[rows, :].T),
            "wvT": np.ascontiguousarray(Wv[hg * DPC:(hg + 1) * DPC, :].T),
            "woT": np.ascontiguousarray(Wo[:, hg * DPC:(hg + 1) * DPC].T),
            "pos": pos_np,
            "invf": invf_np,
        })

    res = run_bass_kernel_spmd(nc, in_maps, list(range(N_CORES)))
    out = np.empty((B, S, D), dtype=np.float32)
    for b in range(B):
        out[b] = res.results[2 * b]["y"] + res.results[2 * b + 1]["y"]
    return out


# revision 20
# speedup vs baseline: 1.8654x; 1.0560x over previous
"""Causal multi-head self-attention with RoPE on 8 Trainium2 NeuronCores.

Sharding: data-parallel over batch (B=4 -> 2 cores per batch) x tensor-parallel
over heads (16 heads -> 8 per core). Each core computes q/k/v projections for
its 8 heads, RoPE, causal attention, and a partial o_proj; the host sums the
two partial o_proj outputs per batch.

Per-core layout strategy:
  - x^T [1024, 2048] and pre-transposed weight shards are DMA'd in (host does
    the pure-layout transposes; all FLOPs run on device). x^T is streamed per
    512-column chunk to fit SBUF.
  - Q^T/K^T are produced head-major ([dk, s]) so attention needs no on-chip
    transposes; scores are computed transposed ([s_k, s_q]) so the softmax
    denominator comes from a ones-column augmented V matmul.
  - Scores matmuls for the two heads sharing a 128-row chunk are packed into
    PE row-groups 0-63/64-127 via tile_position -> both run concurrently.
  - RoPE uses the "evens-then-odds" dk permutation (folded into the weight
    row order on the host), turning interleaved rotation into contiguous
    32-row block rotation; cos/sin tables are built on device from
    token_positions with Cody-Waite range reduction.
  - exp(scores/8) runs on the scalar engine straight out of PSUM; the causal
    mask is an affine_select on the diagonal 128-col block (GPSIMD).
  - Attention runs qc-outermost so each 512-query chunk of o_proj interleaves
    with the next chunk's (ACT-bound) attention.
"""

import sys

sys.path.insert(0, "/opt/trn_rl_repo")

import os as _os

import numpy as np

import concourse.bass as bass
import concourse.tile as tile
from concourse import bacc, mybir
from concourse.bass_utils import run_bass_kernel_spmd
from concourse.masks import make_identity

B, S, D, H = 4, 2048, 1024, 16
DK = D // H            # 64
HPC = H // 2           # 8 heads per core
DPC = HPC * DK         # 512 head dims per core
N_CORES = 8
HALF = DK // 2         # 32 rotary pairs
THETA = 10000.0

AF = mybir.ActivationFunctionType
F32 = mybir.dt.float32
F32R = mybir.dt.float32r
I32 = mybir.dt.int32

# Matmul input dtype for the big GEMMs: float32 = exact (4 cyc/row),
# float32r = TF32 (1 cyc/row at N>=256).
MM_DT = F32R if _os.environ.get("MMDT", "f32r") == "f32r" else F32

TWO_PI = 2.0 * np.pi
# 3-term Cody-Waite split of 2*pi (c1/c2 have short mantissas so k*c is exact)
_CW_C1 = 6.28125
_CW_C2 = float(np.float32(9.67025756835937500e-4))
_CW_C3 = float(TWO_PI - _CW_C1 - np.float32(9.67025756835937500e-4))


def _build_program(debug=False):
    nc = bacc.Bacc("TRN2", target_bir_lowering=False, debug=False)

    xT = nc.dram_tensor("xT", [D, S], F32, kind="ExternalInput").ap()
    wqT = nc.dram_tensor("wqT", [D, DPC], F32, kind="ExternalInput").ap()
    wkT = nc.dram_tensor("wkT", [D, DPC], F32, kind="ExternalInput").ap()
    wvT = nc.dram_tensor("wvT", [D, DPC], F32, kind="ExternalInput").ap()
    woT = nc.dram_tensor("woT", [DPC, D], F32, kind="ExternalInput").ap()
    pos = nc.dram_tensor("pos", [S], I32, kind="ExternalInput").ap()
    invf_in = nc.dram_tensor("invf", [HALF], F32, kind="ExternalInput").ap()
    y = nc.dram_tensor("y", [S, D], F32, kind="ExternalOutput").ap()

    dbg = None
    if debug:
        dbg = {
            "cs_dump": nc.dram_tensor("cs_dump", [2, 128, S], F32, kind="ExternalOutput").ap(),
            "qk_dump": nc.dram_tensor("qk_dump", [128, 8, S], F32, kind="ExternalOutput").ap(),
            "vp_dump": nc.dram_tensor("vp_dump", [128, S // 128, HPC * (DK + 1)], F32, kind="ExternalOutput").ap(),
            "heads_dump": nc.dram_tensor("heads_dump", [128, DPC // 128, S], F32, kind="ExternalOutput").ap(),
            "e_dump": nc.dram_tensor("e_dump", [16, 4, 2, 128, 512], F32, kind="ExternalOutput").ap(),
            "o_dump": nc.dram_tensor("o_dump", [4, 2, DK + 1, 512], F32, kind="ExternalOutput").ap(),
            "recip_dump": nc.dram_tensor("recip_dump", [4, 2, 512], F32, kind="ExternalOutput").ap(),
            "rb_dump": nc.dram_tensor("rb_dump", [4, 2, DK, 512], F32, kind="ExternalOutput").ap(),
            "hn_dump": nc.dram_tensor("hn_dump", [4, 2, DK, 512], F32, kind="ExternalOutput").ap(),
        }

    with tile.TileContext(nc) as tc:
        _emit(nc, tc, xT, wqT, wkT, wvT, woT, pos, invf_in, y, dbg)

    nc.compile()
    return nc


def _emit(nc, tc, xT, wqT, wkT, wvT, woT, pos, invf_in, y, dbg=None):
    import contextlib

    ctx = contextlib.ExitStack()
    with ctx:
        persist = ctx.enter_context(tc.tile_pool(name="persist", bufs=1))
        p_swap = persist.tile([128, 128], F32)
        identity = persist.tile([128, 128], F32)
        make_identity(nc, identity)
        # P_swap: swap 32-row blocks within each 64-block (rows 0<->32, 64<->96)
        nc.gpsimd.memset(p_swap, 0.0)
        for blk in range(4):
            src = (blk ^ 1) * 32
            nc.sync.dma_start(
                out=p_swap[blk * 32:(blk + 1) * 32, :],
                in_=identity[src:src + 32, :],
            )
        ones_col = persist.tile([128, 1], F32)
        nc.vector.memset(ones_col, 1.0)

        # cbig/sbig [128, S]: 32-row blocks [cos x4] and [-sin; sin; -sin; sin]
        # so RoPE on a [128, s] slice of Q^T/K^T is
        #   q' = q * cbig + (P_swap @ q) * sbig
        cs_pool = tc.alloc_tile_pool(name="cs", bufs=1, side="right")
        cbig = cs_pool.tile([128, S], F32)
        sbig = cs_pool.tile([128, S], F32)

        # ---------------- Phase B: rotary tables ----------------
        with tc.tile_pool(name="tables", bufs=1) as tbl, \
             tc.tile_pool(name="tbl_ps", bufs=1, space="PSUM") as tbl_ps:
            posi = tbl.tile([1, S], I32)
            nc.sync.dma_start(out=posi, in_=pos.unsqueeze(0))
            posf = tbl.tile([1, S], F32)
            nc.vector.tensor_copy(posf, posi)
            invf = tbl.tile([1, HALF], F32)
            nc.sync.dma_start(out=invf, in_=invf_in.unsqueeze(0))

            ang_ps = tbl_ps.tile([HALF, S], F32)
            for j in range(S // 512):
                nc.tensor.matmul(ang_ps[:, j * 512:(j + 1) * 512], invf,
                                 posf[:, j * 512:(j + 1) * 512],
                                 start=True, stop=True)
            ang = tbl.tile([HALF, S], F32)
            nc.scalar.copy(ang, ang_ps)

            k_i = tbl.tile([HALF, S], I32)
            nc.scalar.activation(k_i, ang, AF.Copy, scale=float(1.0 / TWO_PI))
            k_f = tbl.tile([HALF, S], F32)
            nc.vector.tensor_copy(k_f, k_i)
            ang_red = tbl.tile([HALF, S], F32)
            nc.vector.cody_waite_cascade(ang_red, ang, k_f, _CW_C1, _CW_C2, _CW_C3)

            sin_arg = tbl.tile([HALF, S], F32)
            cos_arg = tbl.tile([HALF, S], F32)
            nc.vector.add_range_wrap(sin_arg, ang_red, 0.0, float(np.pi), TWO_PI)
            nc.vector.add_range_wrap(cos_arg, ang_red, float(np.pi / 2), float(np.pi), TWO_PI)

            nc.scalar.activation(cbig[0:HALF, :], cos_arg, AF.Sin)
            s_pos = tbl.tile([HALF, S], F32)
            nc.scalar.activation(s_pos, sin_arg, AF.Sin)
            nc.scalar.mul(sbig[0:HALF, :], s_pos, -1.0)
            nc.sync.dma_start(out=sbig[HALF:2 * HALF, :], in_=s_pos)
            nc.sync.dma_start(out=cbig[HALF:2 * HALF, :], in_=cbig[0:HALF, :])
            nc.sync.dma_start(out=cbig[64:128, :], in_=cbig[0:64, :])
            nc.sync.dma_start(out=sbig[64:128, :], in_=sbig[0:64, :])
            if dbg is not None:
                nc.sync.dma_start(out=dbg["cs_dump"][0], in_=cbig)
                nc.sync.dma_start(out=dbg["cs_dump"][1], in_=sbig)

        # Q^T/K^T head-major, resident: [128, (q|k)*4 + e-chunk, S]
        qkT_pool = ctx.enter_context(tc.tile_pool(name="qkT", bufs=1))
        qkT = qkT_pool.tile([128, 8, S], MM_DT)

        # ---------------- Phase C1: Q/K projections + RoPE ----------------
        with tc.tile_pool(name="wqk", bufs=1) as wqk_pool, \
             tc.tile_pool(name="xts", bufs=2) as xts_pool, \
             tc.tile_pool(name="ctmp", bufs=3) as ctmp, \
             tc.tile_pool(name="ps_c", bufs=4, space="PSUM") as proj_ps, \
             tc.tile_pool(name="ps_w", bufs=2, space="PSUM") as swap_ps:
            w_qk = wqk_pool.tile([128, 2, D // 128, DPC], MM_DT)
            for qk_idx, w_dram in ((0, wqT), (1, wkT)):
                for dc in range(D // 128):
                    eng = nc.sync if (dc % 2 == 0) else nc.scalar
                    eng.dma_start(
                        out=w_qk[:, qk_idx, dc, :],
                        in_=w_dram.bitcast(MM_DT)[dc * 128:(dc + 1) * 128, :])
            for sc in range(S // 512):
                ssl = bass.ts(sc, 512)
                xts = xts_pool.tile([128, D // 128, 512], MM_DT, name="xts")
                for dc in range(D // 128):
                    eng = nc.sync if (dc % 2 == 0) else nc.scalar
                    eng.dma_start(
                        out=xts[:, dc, :],
                        in_=xT.bitcast(MM_DT)[dc * 128:(dc + 1) * 128, ssl])
                for qk_idx in (0, 1):
                    for et in range(DPC // 128):
                        p_t = proj_ps.tile([128, 512], F32, name="p_t")
                        for dc in range(D // 128):
                            nc.tensor.matmul(
                                p_t, w_qk[:, qk_idx, dc, et * 128:(et + 1) * 128],
                                xts[:, dc, :],
                                start=(dc == 0), stop=(dc == D // 128 - 1))
                        qt_sb = ctmp.tile([128, 512], F32, name="qt_sb")
                        nc.scalar.copy(qt_sb, p_t)
                        # RoPE: out = qt*C + (P_swap @ qt)*S
                        sw_ps = swap_ps.tile([128, 512], F32, name="sw_ps")
                        nc.tensor.matmul(sw_ps, p_swap, qt_sb, start=True, stop=True)
                        g1 = ctmp.tile([128, 512], F32, name="g1")
                        nc.gpsimd.tensor_mul(g1, qt_sb, cbig[:, ssl])
                        d1 = ctmp.tile([128, 512], F32, name="d1")
                        nc.vector.tensor_mul(d1, sw_ps, sbig[:, ssl])
                        qtr = ctmp.tile([128, 512], F32, name="qtr")
                        nc.vector.tensor_add(qtr, g1, d1)
                        nc.sync.dma_start(out=qkT[:, qk_idx * 4 + et, ssl],
                                          in_=qtr.bitcast(MM_DT))
        cs_pool.release()

        # ---------------- Phase C2: V projection ----------------
        vp_pool = ctx.enter_context(tc.tile_pool(name="vp", bufs=1))
        vp = vp_pool.tile([128, S // 128, HPC * (DK + 1)], MM_DT)
        vp_heads = vp.rearrange("p s (h c) -> p s h c", h=HPC)
        nc.scalar.copy(vp_heads[:, :, :, DK:DK + 1],
                       ones_col.to_broadcast((128, S // 128, HPC, 1)))
        with tc.tile_pool(name="wv", bufs=1) as wv_pool, \
             tc.tile_pool(name="xts2", bufs=2) as xts2_pool, \
             tc.tile_pool(name="ps_v", bufs=4, space="PSUM") as v_ps:
            wv_t = wv_pool.tile([128, D // 128, DPC], MM_DT)
            for dc in range(D // 128):
                eng = nc.sync if (dc % 2 == 0) else nc.scalar
                eng.dma_start(out=wv_t[:, dc, :],
                              in_=wvT.bitcast(MM_DT)[dc * 128:(dc + 1) * 128, :])
            for sc in range(S // 512):
                xts = xts2_pool.tile([128, D // 128, 512], MM_DT, name="xts2")
                for dc in range(D // 128):
                    eng = nc.sync if (dc % 2 == 0) else nc.scalar
                    eng.dma_start(
                        out=xts[:, dc, :],
                        in_=xT.bitcast(MM_DT)[dc * 128:(dc + 1) * 128, bass.ts(sc, 512)])
                for st4 in range(4):
                    pv_t = v_ps.tile([128, 512], F32, name="pv_t")
                    for dc in range(D // 128):
                        nc.tensor.matmul(
                            pv_t, xts[:, dc, st4 * 128:(st4 + 1) * 128], wv_t[:, dc, :],
                            start=(dc == 0), stop=(dc == D // 128 - 1))
                    nc.scalar.copy(vp_heads[:, sc * 4 + st4, :, 0:DK], pv_t)
        if dbg is not None:
            nc.sync.dma_start(out=dbg["vp_dump"], in_=vp.bitcast(F32))
            nc.sync.dma_start(out=dbg["qk_dump"], in_=qkT.bitcast(F32))

        # ---------------- Phase D: attention + interleaved o_proj ----------
        heads_pool = ctx.enter_context(tc.tile_pool(name="heads", bufs=1))
        heads_t = heads_pool.tile([128, DPC // 128, S], MM_DT)
        wo_pool = ctx.enter_context(tc.tile_pool(name="wo", bufs=1))
        wo_t = wo_pool.tile([128, DPC // 128, D], MM_DT)
        for dc in range(DPC // 128):
            eng = nc.sync if (dc % 2 == 0) else nc.scalar
            eng.dma_start(out=wo_t[:, dc, :],
                          in_=woT.bitcast(MM_DT)[dc * 128:(dc + 1) * 128, :])

        with tc.tile_pool(name="expp", bufs=6) as exp_pool, \
             tc.tile_pool(name="norm", bufs=3) as norm_pool, \
             tc.tile_pool(name="yout", bufs=3) as ypool, \
             tc.tile_pool(name="ps_s", bufs=2, space="PSUM") as s_ps, \
             tc.tile_pool(name="ps_o", bufs=1, space="PSUM") as o_ps, \
             tc.tile_pool(name="ps_y", bufs=2, space="PSUM") as y_ps:
            for qc in range(S // 512):
                n_kt = 4 * qc + 4
                for hp in range(HPC // 2):
                    hA, hB = 2 * hp, 2 * hp + 1
                    o_ts = [o_ps.tile([DK + 1, 512], F32, name=f"o_t{ab}")
                            for ab in "AB"]
                    e_ts = [None, None]
                    for kt in range(n_kt):
                        diag = (kt // 4 == qc)
                        # diagonal-chunk strips only cover q >= 128*kt; the
                        # first 128 cols of that range get the triangle mask
                        co = 128 * (kt % 4) if diag else 0
                        n = 512 - co
                        ktsl = bass.ts(kt, 128)
                        qsl = bass.ds(qc * 512 + co, n)
                        for i, (ro, tp) in enumerate(((0, (0, 0)), (64, (64, 0)))):
                            sc_t = s_ps.tile([128, 512], F32, name=f"sc_t{i}")
                            nc.tensor.matmul(
                                sc_t[:, 0:n],
                                qkT[ro:ro + 64, 4 + hp, ktsl],
                                qkT[ro:ro + 64, hp, qsl],
                                start=True, stop=True, tile_position=tp)
                            e_t = exp_pool.tile([128, 512], MM_DT, name=f"e_t{i}")
                            nc.scalar.activation(e_t[:, 0:n], sc_t[:, 0:n], AF.Exp,
                                                 scale=float(1.0 / np.sqrt(DK)))
                            if diag:
                                nc.gpsimd.affine_select(
                                    out=e_t[:, 0:128], in_=e_t[:, 0:128],
                                    pattern=[[1, 128]], base=0, channel_multiplier=-1,
                                    compare_op=mybir.AluOpType.is_ge, fill=0.0)
                            e_ts[i] = e_t
                            if dbg is not None and hp == 0:
                                nc.sync.dma_start(
                                    out=dbg["e_dump"][kt, qc, i, :, 0:n],
                                    in_=e_t.bitcast(F32)[:, 0:n])
                        for i, h in enumerate((hA, hB)):
                            nc.tensor.matmul(
                                o_ts[i][:, co:512],
                                vp[:, kt, h * (DK + 1):(h + 1) * (DK + 1)],
                                e_ts[i][:, 0:n],
                                start=(kt == 0), stop=(kt == n_kt - 1))
                    for i in range(2):
                        ob = norm_pool.tile([DK + 1, 512], F32, name="ob")
                        nc.vector.tensor_copy(ob, o_ts[i])
                        if dbg is not None and hp == 0:
                            nc.sync.dma_start(out=dbg["o_dump"][qc, i], in_=ob)
                        dsb = norm_pool.tile([1, 512], F32, name="dsb")
                        nc.vector.tensor_copy(dsb, ob[DK:DK + 1, :])
                        recip = norm_pool.tile([1, 512], F32, name="recip")
                        nc.vector.reciprocal_approx_fast(recip, dsb)
                        rb = norm_pool.tile([DK, 512], F32, name="rb")
                        nc.gpsimd.partition_broadcast(rb, recip)
                        hn = norm_pool.tile([DK, 512], F32, name="hn")
                        nc.vector.tensor_mul(hn, ob[0:DK, :], rb)
                        if dbg is not None and hp == 0:
                            nc.sync.dma_start(out=dbg["recip_dump"][qc, i].unsqueeze(0), in_=recip)
                            nc.sync.dma_start(out=dbg["rb_dump"][qc, i], in_=rb)
                            nc.sync.dma_start(out=dbg["hn_dump"][qc, i], in_=hn)
                        nc.sync.dma_start(
                            out=heads_t[i * DK:(i + 1) * DK, hp, bass.ts(qc, 512)],
                            in_=hn.bitcast(MM_DT))
                # o_proj for this query chunk (heads_t[:, :, qc*512:...] ready)
                for st4 in range(4):
                    st = qc * 4 + st4
                    for nb in range(D // 512):
                        py_t = y_ps.tile([128, 512], F32, name="py_t")
                        for dc in range(DPC // 128):
                            nc.tensor.matmul(
                                py_t, heads_t[:, dc, st * 128:(st + 1) * 128],
                                wo_t[:, dc, bass.ts(nb, 512)],
                                start=(dc == 0), stop=(dc == DPC // 128 - 1))
                        y_sb = ypool.tile([128, 512], F32, name="y_sb")
                        nc.vector.tensor_copy(y_sb, py_t)
                        nc.sync.dma_start(
                            out=y[st * 128:(st + 1) * 128, bass.ts(nb, 512)], in_=y_sb)

        if dbg is not None:
            nc.sync.dma_start(out=dbg["heads_dump"], in_=heads_t.bitcast(F32))


def _host_inv_freq():
    import jax
    import jax.numpy as jnp
    with jax.default_device(jax.devices("cpu")[0]):
        v = 1.0 / (THETA ** (jnp.arange(HALF, dtype=jnp.float32) * 2.0 / DK))
        return np.asarray(jax.device_get(v)).astype(np.float32)


_program_cache = None


def _get_program():
    global _program_cache
    if _program_cache is None:
        _program_cache = _build_program()
    return _program_cache


# dk permutation: evens then odds within each head's 64 dims
_PERM64 = np.concatenate([np.arange(0, DK, 2), np.arange(1, DK, 2)])


def _make_in_maps(x, Wq, Wk, Wv, Wo, pos_np):
    invf_np = _host_inv_freq()
    in_maps = []
    for c in range(N_CORES):
        b, hg = c // 2, c % 2
        rows = hg * DPC + np.concatenate(
            [h * DK + _PERM64 for h in range(HPC)])
        in_maps.append({
            "xT": np.ascontiguousarray(x[b].T),
            "wqT": np.ascontiguousarray(Wq[rows, :].T),
            "wkT": np.ascontiguousarray(Wk[rows, :].T),
            "wvT": np.ascontiguousarray(Wv[hg * DPC:(hg + 1) * DPC, :].T),
            "woT": np.ascontiguousarray(Wo[:, hg * DPC:(hg + 1) * DPC].T),
            "pos": pos_np,
            "invf": invf_np,
        })
    return in_maps


def kernel(x, Wq, Wk, Wv, Wo, token_positions):
    x = np.asarray(x, dtype=np.float32)
    Wq = np.asarray(Wq, dtype=np.float32)
    Wk = np.asarray(Wk, dtype=np.float32)
    Wv = np.asarray(Wv, dtype=np.float32)
    Wo = np.asarray(Wo, dtype=np.float32)
    pos_np = np.ascontiguousarray(np.asarray(token_positions, dtype=np.int32))

    nc = _get_program()
    in_maps = _make_in_maps(x, Wq, Wk, Wv, Wo, pos_np)
    res = run_bass_kernel_spmd(nc, in_maps, list(range(N_CORES)))
    out = np.empty((B, S, D), dtype=np.float32)
    for b in range(B):
        out[b] = res.results[2 * b]["y"] + res.results[2 * b + 1]["y"]
    return out


# revision 22
# speedup vs baseline: 2.3077x; 1.2371x over previous
"""Causal multi-head self-attention with RoPE on 8 Trainium2 NeuronCores.

Sharding: data-parallel over batch (B=4 -> 2 cores per batch) x tensor-parallel
over heads (16 heads -> 8 per core). Each core computes q/k/v projections for
its 8 heads, RoPE, causal attention, and a partial o_proj; the host sums the
two partial o_proj outputs per batch.

Per-core layout strategy:
  - x^T [1024, 2048] and pre-transposed weight shards are DMA'd in (host does
    the pure-layout transposes; all FLOPs run on device). x^T is streamed per
    512-column chunk to fit SBUF.
  - Q^T/K^T are produced head-major ([dk, s]) so attention needs no on-chip
    transposes; scores are computed transposed ([s_k, s_q]) so the softmax
    denominator comes from a ones-column augmented V matmul.
  - Scores matmuls for the two heads sharing a 128-row chunk are packed into
    PE row-groups 0-63/64-127 via tile_position -> both run concurrently.
  - RoPE uses the "evens-then-odds" dk permutation (folded into the weight
    row order on the host), turning interleaved rotation into contiguous
    32-row block rotation; cos/sin tables are built on device from
    token_positions with Cody-Waite range reduction.
  - exp(scores/8) runs on the scalar engine straight out of PSUM; the causal
    mask is an affine_select on the diagonal 128-col block (GPSIMD).
  - Attention runs qc-outermost so each 512-query chunk of o_proj interleaves
    with the next chunk's (ACT-bound) attention.
"""

import sys

sys.path.insert(0, "/opt/trn_rl_repo")

import os as _os

import numpy as np

import concourse.bass as bass
import concourse.tile as tile
from concourse import bacc, mybir
from concourse.bass_utils import run_bass_kernel_spmd
from concourse.masks import make_identity

B, S, D, H = 4, 2048, 1024, 16
DK = D // H            # 64
HPC = H // 2           # 8 heads per core
DPC = HPC * DK         # 512 head dims per core
N_CORES = 8
HALF = DK // 2         # 32 rotary pairs
THETA = 10000.0

AF = mybir.ActivationFunctionType
F32 = mybir.dt.float32
F32R = mybir.dt.float32r
I32 = mybir.dt.int32

# Matmul input dtype for the big GEMMs: float32 = exact (4 cyc/row),
# float32r = TF32 (1 cyc/row at N>=256).
MM_DT = F32R if _os.environ.get("MMDT", "f32r") == "f32r" else F32

TWO_PI = 2.0 * np.pi
# 3-term Cody-Waite split of 2*pi (c1/c2 have short mantissas so k*c is exact)
_CW_C1 = 6.28125
_CW_C2 = float(np.float32(9.67025756835937500e-4))
_CW_C3 = float(TWO_PI - _CW_C1 - np.float32(9.67025756835937500e-4))


def _build_program(debug=False):
    nc = bacc.Bacc("TRN2", target_bir_lowering=False, debug=False)

    xT = nc.dram_tensor("xT", [D, S], F32, kind="ExternalInput").ap()
    wqT = nc.dram_tensor("wqT", [D, DPC], F32, kind="ExternalInput").ap()
    wkT = nc.dram_tensor("wkT", [D, DPC], F32, kind="ExternalInput").ap()
    wvT = nc.dram_tensor("wvT", [D, DPC], F32, kind="ExternalInput").ap()
    woT = nc.dram_tensor("woT", [DPC, D], F32, kind="ExternalInput").ap()
    pos = nc.dram_tensor("pos", [S], I32, kind="ExternalInput").ap()
    invf_in = nc.dram_tensor("invf", [HALF], F32, kind="ExternalInput").ap()
    y = nc.dram_tensor("y", [S, D], F32, kind="ExternalOutput").ap()

    dbg = None
    if debug:
        dbg = {
            "cs_dump": nc.dram_tensor("cs_dump", [2, 128, S], F32, kind="ExternalOutput").ap(),
            "qk_dump": nc.dram_tensor("qk_dump", [128, 8, S], F32, kind="ExternalOutput").ap(),
            "vp_dump": nc.dram_tensor("vp_dump", [128, S // 128, HPC * (DK + 1)], F32, kind="ExternalOutput").ap(),
            "heads_dump": nc.dram_tensor("heads_dump", [128, DPC // 128, S], F32, kind="ExternalOutput").ap(),
            "e_dump": nc.dram_tensor("e_dump", [16, 4, 2, 128, 512], F32, kind="ExternalOutput").ap(),
            "o_dump": nc.dram_tensor("o_dump", [4, 2, DK + 1, 512], F32, kind="ExternalOutput").ap(),
            "recip_dump": nc.dram_tensor("recip_dump", [4, 2, 512], F32, kind="ExternalOutput").ap(),
            "rb_dump": nc.dram_tensor("rb_dump", [4, 2, DK, 512], F32, kind="ExternalOutput").ap(),
            "hn_dump": nc.dram_tensor("hn_dump", [4, 2, DK, 512], F32, kind="ExternalOutput").ap(),
        }

    with tile.TileContext(nc) as tc:
        _emit(nc, tc, xT, wqT, wkT, wvT, woT, pos, invf_in, y, dbg)

    nc.compile()
    return nc


def _emit(nc, tc, xT, wqT, wkT, wvT, woT, pos, invf_in, y, dbg=None):
    import contextlib

    ctx = contextlib.ExitStack()
    with ctx:
        persist = ctx.enter_context(tc.tile_pool(name="persist", bufs=1))
        p_swap = persist.tile([128, 128], F32)
        identity = persist.tile([128, 128], F32)
        make_identity(nc, identity)
        # P_swap: swap 32-row blocks within each 64-block (rows 0<->32, 64<->96)
        nc.gpsimd.memset(p_swap, 0.0)
        for blk in range(4):
            src = (blk ^ 1) * 32
            nc.sync.dma_start(
                out=p_swap[blk * 32:(blk + 1) * 32, :],
                in_=identity[src:src + 32, :],
            )
        ones_col = persist.tile([128, 1], F32)
        nc.vector.memset(ones_col, 1.0)

        # cbig/sbig [128, S]: 32-row blocks [cos x4] and [-sin; sin; -sin; sin]
        # so RoPE on a [128, s] slice of Q^T/K^T is
        #   q' = q * cbig + (P_swap @ q) * sbig
        cs_pool = tc.alloc_tile_pool(name="cs", bufs=1, side="right")
        cbig = cs_pool.tile([128, S], F32)
        sbig = cs_pool.tile([128, S], F32)

        # ---------------- Phase B: rotary tables ----------------
        with tc.tile_pool(name="tables", bufs=1) as tbl, \
             tc.tile_pool(name="tbl_ps", bufs=1, space="PSUM") as tbl_ps:
            posi = tbl.tile([1, S], I32)
            nc.sync.dma_start(out=posi, in_=pos.unsqueeze(0))
            posf = tbl.tile([1, S], F32)
            nc.vector.tensor_copy(posf, posi)
            invf = tbl.tile([1, HALF], F32)
            nc.sync.dma_start(out=invf, in_=invf_in.unsqueeze(0))

            ang_ps = tbl_ps.tile([HALF, S], F32)
            for j in range(S // 512):
                nc.tensor.matmul(ang_ps[:, j * 512:(j + 1) * 512], invf,
                                 posf[:, j * 512:(j + 1) * 512],
                                 start=True, stop=True)
            ang = tbl.tile([HALF, S], F32)
            nc.scalar.copy(ang, ang_ps)

            k_i = tbl.tile([HALF, S], I32)
            nc.scalar.activation(k_i, ang, AF.Copy, scale=float(1.0 / TWO_PI))
            k_f = tbl.tile([HALF, S], F32)
            nc.vector.tensor_copy(k_f, k_i)
            ang_red = tbl.tile([HALF, S], F32)
            nc.vector.cody_waite_cascade(ang_red, ang, k_f, _CW_C1, _CW_C2, _CW_C3)

            sin_arg = tbl.tile([HALF, S], F32)
            cos_arg = tbl.tile([HALF, S], F32)
            nc.vector.add_range_wrap(sin_arg, ang_red, 0.0, float(np.pi), TWO_PI)
            nc.vector.add_range_wrap(cos_arg, ang_red, float(np.pi / 2), float(np.pi), TWO_PI)

            nc.scalar.activation(cbig[0:HALF, :], cos_arg, AF.Sin)
            s_pos = tbl.tile([HALF, S], F32)
            nc.scalar.activation(s_pos, sin_arg, AF.Sin)
            nc.scalar.mul(sbig[0:HALF, :], s_pos, -1.0)
            nc.sync.dma_start(out=sbig[HALF:2 * HALF, :], in_=s_pos)
            nc.sync.dma_start(out=cbig[HALF:2 * HALF, :], in_=cbig[0:HALF, :])
            nc.sync.dma_start(out=cbig[64:128, :], in_=cbig[0:64, :])
            nc.sync.dma_start(out=sbig[64:128, :], in_=sbig[0:64, :])
            if dbg is not None:
                nc.sync.dma_start(out=dbg["cs_dump"][0], in_=cbig)
                nc.sync.dma_start(out=dbg["cs_dump"][1], in_=sbig)

        # Q^T/K^T head-major, resident: [128, (q|k)*4 + e-chunk, S]
        qkT_pool = ctx.enter_context(tc.tile_pool(name="qkT", bufs=1))
        qkT = qkT_pool.tile([128, 8, S], MM_DT)

        # ---------------- Phase C1: Q/K projections + RoPE ----------------
        with tc.tile_pool(name="wqk", bufs=1) as wqk_pool, \
             tc.tile_pool(name="xts", bufs=3) as xts_pool, \
             tc.tile_pool(name="ctmp", bufs=3) as ctmp, \
             tc.tile_pool(name="ps_c", bufs=4, space="PSUM") as proj_ps, \
             tc.tile_pool(name="ps_w", bufs=2, space="PSUM") as swap_ps:
            w_qk = wqk_pool.tile([128, 2, D // 128, DPC], MM_DT)
            for qk_idx, w_dram in ((0, wqT), (1, wkT)):
                for dc in range(D // 128):
                    eng = nc.sync if (dc % 2 == 0) else nc.scalar
                    eng.dma_start(
                        out=w_qk[:, qk_idx, dc, :],
                        in_=w_dram.bitcast(MM_DT)[dc * 128:(dc + 1) * 128, :])
            for sc in range(S // 512):
                ssl = bass.ts(sc, 512)
                xts = xts_pool.tile([128, D // 128, 512], MM_DT, name="xts")
                for dc in range(D // 128):
                    eng = nc.sync if (dc % 2 == 0) else nc.scalar
                    eng.dma_start(
                        out=xts[:, dc, :],
                        in_=xT.bitcast(MM_DT)[dc * 128:(dc + 1) * 128, ssl])
                for qk_idx in (0, 1):
                    for et in range(DPC // 128):
                        p_t = proj_ps.tile([128, 512], F32, name="p_t")
                        for dc in range(D // 128):
                            nc.tensor.matmul(
                                p_t, w_qk[:, qk_idx, dc, et * 128:(et + 1) * 128],
                                xts[:, dc, :],
                                start=(dc == 0), stop=(dc == D // 128 - 1))
                        qt_sb = ctmp.tile([128, 512], F32, name="qt_sb")
                        nc.scalar.copy(qt_sb, p_t)
                        # RoPE: out = qt*C + (P_swap @ qt)*S
                        sw_ps = swap_ps.tile([128, 512], F32, name="sw_ps")
                        nc.tensor.matmul(sw_ps, p_swap, qt_sb, start=True, stop=True)
                        g1 = ctmp.tile([128, 512], F32, name="g1")
                        nc.gpsimd.tensor_mul(g1, qt_sb, cbig[:, ssl])
                        d1 = ctmp.tile([128, 512], F32, name="d1")
                        nc.vector.tensor_mul(d1, sw_ps, sbig[:, ssl])
                        qtr = ctmp.tile([128, 512], F32, name="qtr")
                        nc.vector.tensor_add(qtr, g1, d1)
                        nc.sync.dma_start(out=qkT[:, qk_idx * 4 + et, ssl],
                                          in_=qtr.bitcast(MM_DT))
        cs_pool.release()

        # ---------------- Phase C2: V projection ----------------
        vp_pool = ctx.enter_context(tc.tile_pool(name="vp", bufs=1))
        vp = vp_pool.tile([128, S // 128, HPC * (DK + 1)], MM_DT)
        vp_heads = vp.rearrange("p s (h c) -> p s h c", h=HPC)
        nc.scalar.copy(vp_heads[:, :, :, DK:DK + 1],
                       ones_col.to_broadcast((128, S // 128, HPC, 1)))
        with tc.tile_pool(name="wv", bufs=1) as wv_pool, \
             tc.tile_pool(name="xts2", bufs=3) as xts2_pool, \
             tc.tile_pool(name="ps_v", bufs=4, space="PSUM") as v_ps:
            wv_t = wv_pool.tile([128, D // 128, DPC], MM_DT)
            for dc in range(D // 128):
                eng = nc.sync if (dc % 2 == 0) else nc.scalar
                eng.dma_start(out=wv_t[:, dc, :],
                              in_=wvT.bitcast(MM_DT)[dc * 128:(dc + 1) * 128, :])
            for sc in range(S // 512):
                xts = xts2_pool.tile([128, D // 128, 512], MM_DT, name="xts2")
                for dc in range(D // 128):
                    eng = nc.sync if (dc % 2 == 0) else nc.scalar
                    eng.dma_start(
                        out=xts[:, dc, :],
                        in_=xT.bitcast(MM_DT)[dc * 128:(dc + 1) * 128, bass.ts(sc, 512)])
                for st4 in range(4):
                    pv_t = v_ps.tile([128, 512], F32, name="pv_t")
                    for dc in range(D // 128):
                        nc.tensor.matmul(
                            pv_t, xts[:, dc, st4 * 128:(st4 + 1) * 128], wv_t[:, dc, :],
                            start=(dc == 0), stop=(dc == D // 128 - 1))
                    nc.scalar.copy(vp_heads[:, sc * 4 + st4, :, 0:DK], pv_t)
        if dbg is not None:
            nc.sync.dma_start(out=dbg["vp_dump"], in_=vp.bitcast(F32))
            nc.sync.dma_start(out=dbg["qk_dump"], in_=qkT.bitcast(F32))

        # ---------------- Phase D: attention + interleaved o_proj ----------
        heads_pool = ctx.enter_context(tc.tile_pool(name="heads", bufs=1))
        heads_t = heads_pool.tile([128, DPC // 128, S], MM_DT)
        wo_pool = ctx.enter_context(tc.tile_pool(name="wo", bufs=1))
        wo_t = wo_pool.tile([128, DPC // 128, D], MM_DT)
        for dc in range(DPC // 128):
            eng = nc.sync if (dc % 2 == 0) else nc.scalar
            eng.dma_start(out=wo_t[:, dc, :],
                          in_=woT.bitcast(MM_DT)[dc * 128:(dc + 1) * 128, :])

        with tc.tile_pool(name="expp", bufs=6) as exp_pool, \
             tc.tile_pool(name="norm", bufs=3) as norm_pool, \
             tc.tile_pool(name="yout", bufs=3) as ypool, \
             tc.tile_pool(name="ps_s", bufs=2, space="PSUM") as s_ps, \
             tc.tile_pool(name="ps_o", bufs=1, space="PSUM") as o_ps, \
             tc.tile_pool(name="ps_y", bufs=2, space="PSUM") as y_ps:
            for qc in range(S // 512):
                n_kt = 4 * qc + 4
                for hp in range(HPC // 2):
                    hA, hB = 2 * hp, 2 * hp + 1
                    o_ts = [o_ps.tile([DK + 1, 512], F32, name=f"o_t{ab}")
                            for ab in "AB"]

                    def emit_scores(kt):
                        # packed pair: head A on PE row-group 0-1, head B on
                        # 2-3; both halves land in one 2-bank psum tile so a
                        # single wide exp covers them
                        diag = (kt // 4 == qc)
                        co = 128 * (kt % 4) if diag else 0
                        n = 512 - co
                        ktsl = bass.ts(kt, 128)
                        qsl = bass.ds(qc * 512 + co, n)
                        sc_t = s_ps.tile([128, 1024], F32, name="sc_t")
                        for i, (ro, tp) in enumerate(((0, (0, 0)), (64, (64, 0)))):
                            nc.tensor.matmul(
                                sc_t[:, i * 512:i * 512 + n],
                                qkT[ro:ro + 64, 4 + hp, ktsl],
                                qkT[ro:ro + 64, hp, qsl],
                                start=True, stop=True, tile_position=tp)
                        e_t = exp_pool.tile([128, 1024], MM_DT, name="e_t")
                        if co == 0:
                            nc.scalar.activation(e_t, sc_t, AF.Exp,
                                                 scale=float(1.0 / np.sqrt(DK)))
                        else:
                            for i in range(2):
                                nc.scalar.activation(
                                    e_t[:, i * 512:i * 512 + n],
                                    sc_t[:, i * 512:i * 512 + n], AF.Exp,
                                    scale=float(1.0 / np.sqrt(DK)))
                        if diag:
                            for i in range(2):
                                nc.gpsimd.affine_select(
                                    out=e_t[:, i * 512:i * 512 + 128],
                                    in_=e_t[:, i * 512:i * 512 + 128],
                                    pattern=[[1, 128]], base=0, channel_multiplier=-1,
                                    compare_op=mybir.AluOpType.is_ge, fill=0.0)
                        if dbg is not None and hp == 0:
                            for i in range(2):
                                nc.sync.dma_start(
                                    out=dbg["e_dump"][kt, qc, i, :, 0:n],
                                    in_=e_t.bitcast(F32)[:, i * 512:i * 512 + n])
                        return e_t

                    def emit_av(kt, e_t):
                        diag = (kt // 4 == qc)
                        co = 128 * (kt % 4) if diag else 0
                        n = 512 - co
                        for i, h in enumerate((hA, hB)):
                            nc.tensor.matmul(
                                o_ts[i][:, co:512],
                                vp[:, kt, h * (DK + 1):(h + 1) * (DK + 1)],
                                e_t[:, i * 512:i * 512 + n],
                                start=(kt == 0), stop=(kt == n_kt - 1))

                    # software pipeline: scores(kt+1) is emitted before av(kt)
                    # so the packed score pair stays adjacent in the PE queue
                    prev_e = emit_scores(0)
                    for kt in range(1, n_kt):
                        e_t = emit_scores(kt)
                        emit_av(kt - 1, prev_e)
                        prev_e = e_t
                    emit_av(n_kt - 1, prev_e)
                    for i in range(2):
                        ob = norm_pool.tile([DK + 1, 512], F32, name="ob")
                        nc.vector.tensor_copy(ob, o_ts[i])
                        if dbg is not None and hp == 0:
                            nc.sync.dma_start(out=dbg["o_dump"][qc, i], in_=ob)
                        dsb = norm_pool.tile([1, 512], F32, name="dsb")
                        nc.vector.tensor_copy(dsb, ob[DK:DK + 1, :])
                        recip = norm_pool.tile([1, 512], F32, name="recip")
                        nc.vector.reciprocal_approx_fast(recip, dsb)
                        rb = norm_pool.tile([DK, 512], F32, name="rb")
                        nc.gpsimd.partition_broadcast(rb, recip)
                        hn = norm_pool.tile([DK, 512], F32, name="hn")
                        nc.vector.tensor_mul(hn, ob[0:DK, :], rb)
                        if dbg is not None and hp == 0:
                            nc.sync.dma_start(out=dbg["recip_dump"][qc, i].unsqueeze(0), in_=recip)
                            nc.sync.dma_start(out=dbg["rb_dump"][qc, i], in_=rb)
                            nc.sync.dma_start(out=dbg["hn_dump"][qc, i], in_=hn)
                        nc.sync.dma_start(
                            out=heads_t[i * DK:(i + 1) * DK, hp, bass.ts(qc, 512)],
                            in_=hn.bitcast(MM_DT))
                # o_proj for this query chunk (heads_t[:, :, qc*512:...] ready)
                for st4 in range(4):
                    st = qc * 4 + st4
                    for nb in range(D // 512):
                        py_t = y_ps.tile([128, 512], F32, name="py_t")
                        for dc in range(DPC // 128):
                            nc.tensor.matmul(
                                py_t, heads_t[:, dc, st * 128:(st + 1) * 128],
                                wo_t[:, dc, bass.ts(nb, 512)],
                                start=(dc == 0), stop=(dc == DPC // 128 - 1))
                        y_sb = ypool.tile([128, 512], F32, name="y_sb")
                        nc.vector.tensor_copy(y_sb, py_t)
                        nc.sync.dma_start(
                            out=y[st * 128:(st + 1) * 128, bass.ts(nb, 512)], in_=y_sb)

        if dbg is not None:
            nc.sync.dma_start(out=dbg["heads_dump"], in_=heads_t.bitcast(F32))


def _host_inv_freq():
    import jax
    import jax.numpy as jnp
    with jax.default_device(jax.devices("cpu")[0]):
        v = 1.0 / (THETA ** (jnp.arange(HALF, dtype=jnp.float32) * 2.0 / DK))
        return np.asarray(jax.device_get(v)).astype(np.float32)


_program_cache = None


def _get_program():
    global _program_cache
    if _program_cache is None:
        _program_cache = _build_program()
    return _program_cache


# dk permutation: evens then odds within each head's 64 dims
_PERM64 = np.concatenate([np.arange(0, DK, 2), np.arange(1, DK, 2)])


def _make_in_maps(x, Wq, Wk, Wv, Wo, pos_np):
    invf_np = _host_inv_freq()
    in_maps = []
    for c in range(N_CORES):
        b, hg = c // 2, c % 2
        rows = hg * DPC + np.concatenate(
            [h * DK + _PERM64 for h in range(HPC)])
        in_maps.append({
            "xT": np.ascontiguousarray(x[b].T),
            "wqT": np.ascontiguousarray(Wq[rows, :].T),
            "wkT": np.ascontiguousarray(Wk[rows, :].T),
            "wvT": np.ascontiguousarray(Wv[hg * DPC:(hg + 1) * DPC, :].T),
            "woT": np.ascontiguousarray(Wo[:, hg * DPC:(hg + 1) * DPC].T),
            "pos": pos_np,
            "invf": invf_np,
        })
    return in_maps


def kernel(x, Wq, Wk, Wv, Wo, token_positions):
    x = np.asarray(x, dtype=np.float32)
    Wq = np.asarray(Wq, dtype=np.float32)
    Wk = np.asarray(Wk, dtype=np.float32)
    Wv = np.asarray(Wv, dtype=np.float32)
    Wo = np.asarray(Wo, dtype=np.float32)
    pos_np = np.ascontiguousarray(np.asarray(token_positions, dtype=np.int32))

    nc = _get_program()
    in_maps = _make_in_maps(x, Wq, Wk, Wv, Wo, pos_np)
    res = run_bass_kernel_spmd(nc, in_maps, list(range(N_CORES)))
    out = np.empty((B, S, D), dtype=np.float32)
    for b in range(B):
        out[b] = res.results[2 * b]["y"] + res.results[2 * b + 1]["y"]
    return out


# revision 23
# speedup vs baseline: 2.5485x; 1.1043x over previous
"""Causal multi-head self-attention with RoPE on 8 Trainium2 NeuronCores.

Sharding: data-parallel over batch (B=4 -> 2 cores per batch) x tensor-parallel
over heads (16 heads -> 8 per core). Each core computes q/k/v projections for
its 8 heads, RoPE, causal attention, and a partial o_proj; the host sums the
two partial o_proj outputs per batch.

Per-core layout strategy:
  - x^T [1024, 2048] and pre-transposed weight shards are DMA'd in (host does
    the pure-layout transposes; all FLOPs run on device). x^T is streamed per
    512-column chunk to fit SBUF.
  - Q^T/K^T are produced head-major ([dk, s]) so attention needs no on-chip
    transposes; scores are computed transposed ([s_k, s_q]) so the softmax
    denominator comes from a ones-column augmented V matmul.
  - Scores matmuls for the two heads sharing a 128-row chunk are packed into
    PE row-groups 0-63/64-127 via tile_position -> both run concurrently.
  - RoPE uses the "evens-then-odds" dk permutation (folded into the weight
    row order on the host), turning interleaved rotation into contiguous
    32-row block rotation; cos/sin tables are built on device from
    token_positions with Cody-Waite range reduction.
  - exp(scores/8) runs on the scalar engine straight out of PSUM; the causal
    mask is an affine_select on the diagonal 128-col block (GPSIMD).
  - Attention runs qc-outermost so each 512-query chunk of o_proj interleaves
    with the next chunk's (ACT-bound) attention.
"""

import sys

sys.path.insert(0, "/opt/trn_rl_repo")

import os as _os

import numpy as np

import concourse.bass as bass
import concourse.tile as tile
from concourse import bacc, mybir
from concourse.bass_utils import run_bass_kernel_spmd
from concourse.masks import make_identity

B, S, D, H = 4, 2048, 1024, 16
DK = D // H            # 64
HPC = H // 2           # 8 heads per core
DPC = HPC * DK         # 512 head dims per core
N_CORES = 8
HALF = DK // 2         # 32 rotary pairs
THETA = 10000.0

AF = mybir.ActivationFunctionType
F32 = mybir.dt.float32
F32R = mybir.dt.float32r
I32 = mybir.dt.int32

# Matmul input dtype for the big GEMMs: float32 = exact (4 cyc/row),
# float32r = TF32 (1 cyc/row at N>=256).
MM_DT = F32R if _os.environ.get("MMDT", "f32r") == "f32r" else F32

TWO_PI = 2.0 * np.pi
# 3-term Cody-Waite split of 2*pi (c1/c2 have short mantissas so k*c is exact)
_CW_C1 = 6.28125
_CW_C2 = float(np.float32(9.67025756835937500e-4))
_CW_C3 = float(TWO_PI - _CW_C1 - np.float32(9.67025756835937500e-4))


def _build_program(debug=False):
    nc = bacc.Bacc("TRN2", target_bir_lowering=False, debug=False)

    xT = nc.dram_tensor("xT", [D, S], F32, kind="ExternalInput").ap()
    wqT = nc.dram_tensor("wqT", [D, DPC], F32, kind="ExternalInput").ap()
    wkT = nc.dram_tensor("wkT", [D, DPC], F32, kind="ExternalInput").ap()
    wvT = nc.dram_tensor("wvT", [D, DPC], F32, kind="ExternalInput").ap()
    woT = nc.dram_tensor("woT", [DPC, D], F32, kind="ExternalInput").ap()
    pos = nc.dram_tensor("pos", [S], I32, kind="ExternalInput").ap()
    invf_in = nc.dram_tensor("invf", [HALF], F32, kind="ExternalInput").ap()
    y = nc.dram_tensor("y", [S, D], F32, kind="ExternalOutput").ap()

    dbg = None
    if debug:
        dbg = {
            "cs_dump": nc.dram_tensor("cs_dump", [2, 128, S], F32, kind="ExternalOutput").ap(),
            "qk_dump": nc.dram_tensor("qk_dump", [128, 8, S], F32, kind="ExternalOutput").ap(),
            "vp_dump": nc.dram_tensor("vp_dump", [128, S // 128, HPC * (DK + 1)], F32, kind="ExternalOutput").ap(),
            "heads_dump": nc.dram_tensor("heads_dump", [128, DPC // 128, S], F32, kind="ExternalOutput").ap(),
            "e_dump": nc.dram_tensor("e_dump", [16, 4, 2, 128, 512], F32, kind="ExternalOutput").ap(),
            "o_dump": nc.dram_tensor("o_dump", [4, 2, DK + 1, 512], F32, kind="ExternalOutput").ap(),
            "recip_dump": nc.dram_tensor("recip_dump", [4, 2, 512], F32, kind="ExternalOutput").ap(),
            "rb_dump": nc.dram_tensor("rb_dump", [4, 2, DK, 512], F32, kind="ExternalOutput").ap(),
            "hn_dump": nc.dram_tensor("hn_dump", [4, 2, DK, 512], F32, kind="ExternalOutput").ap(),
        }

    with tile.TileContext(nc) as tc:
        _emit(nc, tc, xT, wqT, wkT, wvT, woT, pos, invf_in, y, dbg)

    nc.compile()
    return nc


def _emit(nc, tc, xT, wqT, wkT, wvT, woT, pos, invf_in, y, dbg=None):
    import contextlib

    ctx = contextlib.ExitStack()
    with ctx:
        persist = ctx.enter_context(tc.tile_pool(name="persist", bufs=1))
        p_swap = persist.tile([128, 128], MM_DT)
        identity = persist.tile([128, 128], F32)
        make_identity(nc, identity)
        # P_swap: swap 32-row blocks within each 64-block (rows 0<->32, 64<->96)
        # (every row of the permuted identity is covered, so no memset needed)
        for blk in range(4):
            src_row = (blk ^ 1) * 32
            nc.sync.dma_start(
                out=p_swap[blk * 32:(blk + 1) * 32, :],
                in_=identity.bitcast(MM_DT)[src_row:src_row + 32, :],
            )
        ones_col = persist.tile([128, 1], F32)
        nc.vector.memset(ones_col, 1.0)

        # cbig/sbig [128, S]: 32-row blocks [cos x4] and [-sin; sin; -sin; sin]
        # so RoPE on a [128, s] slice of Q^T/K^T is
        #   q' = q * cbig + (P_swap @ q) * sbig
        cs_pool = tc.alloc_tile_pool(name="cs", bufs=1, side="right")
        cbig = cs_pool.tile([128, S], F32)
        sbig = cs_pool.tile([128, S], F32)

        # ---------------- Phase B: rotary tables ----------------
        with tc.tile_pool(name="tables", bufs=1) as tbl, \
             tc.tile_pool(name="tbl_ps", bufs=1, space="PSUM") as tbl_ps:
            posi = tbl.tile([1, S], I32)
            nc.sync.dma_start(out=posi, in_=pos.unsqueeze(0))
            posf = tbl.tile([1, S], F32)
            nc.vector.tensor_copy(posf, posi)
            invf = tbl.tile([1, HALF], F32)
            nc.sync.dma_start(out=invf, in_=invf_in.unsqueeze(0))

            ang_ps = tbl_ps.tile([HALF, S], F32)
            for j in range(S // 512):
                nc.tensor.matmul(ang_ps[:, j * 512:(j + 1) * 512], invf,
                                 posf[:, j * 512:(j + 1) * 512],
                                 start=True, stop=True)
            ang = tbl.tile([HALF, S], F32)
            nc.scalar.copy(ang, ang_ps)

            k_i = tbl.tile([HALF, S], I32)
            nc.scalar.activation(k_i, ang, AF.Copy, scale=float(1.0 / TWO_PI))
            k_f = tbl.tile([HALF, S], F32)
            nc.vector.tensor_copy(k_f, k_i)
            ang_red = tbl.tile([HALF, S], F32)
            nc.vector.cody_waite_cascade(ang_red, ang, k_f, _CW_C1, _CW_C2, _CW_C3)

            sin_arg = tbl.tile([HALF, S], F32)
            cos_arg = tbl.tile([HALF, S], F32)
            nc.vector.add_range_wrap(sin_arg, ang_red, 0.0, float(np.pi), TWO_PI)
            nc.vector.add_range_wrap(cos_arg, ang_red, float(np.pi / 2), float(np.pi), TWO_PI)

            nc.scalar.activation(cbig[0:HALF, :], cos_arg, AF.Sin)
            s_pos = tbl.tile([HALF, S], F32)
            nc.scalar.activation(s_pos, sin_arg, AF.Sin)
            nc.scalar.mul(sbig[0:HALF, :], s_pos, -1.0)
            nc.sync.dma_start(out=sbig[HALF:2 * HALF, :], in_=s_pos)
            nc.sync.dma_start(out=cbig[HALF:2 * HALF, :], in_=cbig[0:HALF, :])
            nc.sync.dma_start(out=cbig[64:128, :], in_=cbig[0:64, :])
            nc.sync.dma_start(out=sbig[64:128, :], in_=sbig[0:64, :])
            if dbg is not None:
                nc.sync.dma_start(out=dbg["cs_dump"][0], in_=cbig)
                nc.sync.dma_start(out=dbg["cs_dump"][1], in_=sbig)

        # Q^T/K^T head-major, resident: [128, (q|k)*4 + e-chunk, S]
        qkT_pool = ctx.enter_context(tc.tile_pool(name="qkT", bufs=1))
        qkT = qkT_pool.tile([128, 8, S], MM_DT)

        # ---------------- Phase C1: Q/K projections + RoPE ----------------
        with tc.tile_pool(name="wqk", bufs=1) as wqk_pool, \
             tc.tile_pool(name="xts", bufs=3) as xts_pool, \
             tc.tile_pool(name="ctmp", bufs=3) as ctmp, \
             tc.tile_pool(name="ps_c", bufs=4, space="PSUM") as proj_ps, \
             tc.tile_pool(name="ps_w", bufs=2, space="PSUM") as swap_ps:
            w_qk = wqk_pool.tile([128, 2, D // 128, DPC], MM_DT)
            for qk_idx, w_dram in ((0, wqT), (1, wkT)):
                for dc in range(D // 128):
                    eng = nc.sync if (dc % 2 == 0) else nc.scalar
                    eng.dma_start(
                        out=w_qk[:, qk_idx, dc, :],
                        in_=w_dram.bitcast(MM_DT)[dc * 128:(dc + 1) * 128, :])
            for sc in range(S // 512):
                ssl = bass.ts(sc, 512)
                xts = xts_pool.tile([128, D // 128, 512], MM_DT, name="xts")
                for dc in range(D // 128):
                    eng = nc.sync if (dc % 2 == 0) else nc.scalar
                    eng.dma_start(
                        out=xts[:, dc, :],
                        in_=xT.bitcast(MM_DT)[dc * 128:(dc + 1) * 128, ssl])
                for qk_idx in (0, 1):
                    for et in range(DPC // 128):
                        p_t = proj_ps.tile([128, 512], F32, name="p_t")
                        for dc in range(D // 128):
                            nc.tensor.matmul(
                                p_t, w_qk[:, qk_idx, dc, et * 128:(et + 1) * 128],
                                xts[:, dc, :],
                                start=(dc == 0), stop=(dc == D // 128 - 1))
                        qt_sb = ctmp.tile([128, 512], MM_DT, name="qt_sb")
                        nc.scalar.copy(qt_sb, p_t)
                        # RoPE: out = qt*C + (P_swap @ qt)*S
                        sw_ps = swap_ps.tile([128, 512], F32, name="sw_ps")
                        nc.tensor.matmul(sw_ps, p_swap, qt_sb, start=True, stop=True)
                        g1 = ctmp.tile([128, 512], F32, name="g1")
                        nc.gpsimd.tensor_mul(g1, qt_sb.bitcast(F32), cbig[:, ssl])
                        d1 = ctmp.tile([128, 512], F32, name="d1")
                        nc.vector.tensor_mul(d1, sw_ps, sbig[:, ssl])
                        qtr = ctmp.tile([128, 512], F32, name="qtr")
                        nc.vector.tensor_add(qtr, g1, d1)
                        nc.sync.dma_start(out=qkT[:, qk_idx * 4 + et, ssl],
                                          in_=qtr.bitcast(MM_DT))
        cs_pool.release()

        # ---------------- Phase C2: V projection ----------------
        vp_pool = ctx.enter_context(tc.tile_pool(name="vp", bufs=1))
        vp = vp_pool.tile([128, S // 128, HPC * (DK + 1)], MM_DT)
        vp_heads = vp.rearrange("p s (h c) -> p s h c", h=HPC)
        nc.scalar.copy(vp_heads[:, :, :, DK:DK + 1],
                       ones_col.to_broadcast((128, S // 128, HPC, 1)))
        with tc.tile_pool(name="wv", bufs=1) as wv_pool, \
             tc.tile_pool(name="xts2", bufs=3) as xts2_pool, \
             tc.tile_pool(name="ps_v", bufs=4, space="PSUM") as v_ps:
            wv_t = wv_pool.tile([128, D // 128, DPC], MM_DT)
            for dc in range(D // 128):
                eng = nc.sync if (dc % 2 == 0) else nc.scalar
                eng.dma_start(out=wv_t[:, dc, :],
                              in_=wvT.bitcast(MM_DT)[dc * 128:(dc + 1) * 128, :])
            for sc in range(S // 512):
                xts = xts2_pool.tile([128, D // 128, 512], MM_DT, name="xts2")
                for dc in range(D // 128):
                    eng = nc.sync if (dc % 2 == 0) else nc.scalar
                    eng.dma_start(
                        out=xts[:, dc, :],
                        in_=xT.bitcast(MM_DT)[dc * 128:(dc + 1) * 128, bass.ts(sc, 512)])
                for st4 in range(4):
                    pv_t = v_ps.tile([128, 512], F32, name="pv_t")
                    for dc in range(D // 128):
                        nc.tensor.matmul(
                            pv_t, xts[:, dc, st4 * 128:(st4 + 1) * 128], wv_t[:, dc, :],
                            start=(dc == 0), stop=(dc == D // 128 - 1))
                    nc.scalar.copy(vp_heads[:, sc * 4 + st4, :, 0:DK], pv_t)
        if dbg is not None:
            nc.sync.dma_start(out=dbg["vp_dump"], in_=vp.bitcast(F32))
            nc.sync.dma_start(out=dbg["qk_dump"], in_=qkT.bitcast(F32))

        # ---------------- Phase D: attention + interleaved o_proj ----------
        heads_pool = ctx.enter_context(tc.tile_pool(name="heads", bufs=1))
        heads_t = heads_pool.tile([128, DPC // 128, S], MM_DT)
        wo_pool = ctx.enter_context(tc.tile_pool(name="wo", bufs=1))
        wo_t = wo_pool.tile([128, DPC // 128, D], MM_DT)
        for dc in range(DPC // 128):
            eng = nc.sync if (dc % 2 == 0) else nc.scalar
            eng.dma_start(out=wo_t[:, dc, :],
                          in_=woT.bitcast(MM_DT)[dc * 128:(dc + 1) * 128, :])

        with tc.tile_pool(name="expp", bufs=6) as exp_pool, \
             tc.tile_pool(name="norm", bufs=3) as norm_pool, \
             tc.tile_pool(name="yout", bufs=3) as ypool, \
             tc.tile_pool(name="ps_s", bufs=2, space="PSUM") as s_ps, \
             tc.tile_pool(name="ps_o", bufs=1, space="PSUM") as o_ps, \
             tc.tile_pool(name="ps_y", bufs=2, space="PSUM") as y_ps:
            def emit_oproj_chain(qc, st4, nb):
                st = qc * 4 + st4
                py_t = y_ps.tile([128, 512], F32, name="py_t")
                for dc in range(DPC // 128):
                    nc.tensor.matmul(
                        py_t, heads_t[:, dc, st * 128:(st + 1) * 128],
                        wo_t[:, dc, bass.ts(nb, 512)],
                        start=(dc == 0), stop=(dc == DPC // 128 - 1))
                y_sb = ypool.tile([128, 512], F32, name="y_sb")
                nc.vector.tensor_copy(y_sb, py_t)
                nc.sync.dma_start(
                    out=y[st * 128:(st + 1) * 128, bass.ts(nb, 512)], in_=y_sb)

            for qc in range(S // 512):
                n_kt = 4 * qc + 4
                for hp in range(HPC // 2):
                    # o_proj for the previous query chunk, two chains per
                    # head-pair boundary so they never split a packed score
                    # pair mid-group
                    if qc > 0:
                        for nb in range(2):
                            emit_oproj_chain(qc - 1, hp, nb)
                    hA, hB = 2 * hp, 2 * hp + 1
                    o_ts = [o_ps.tile([DK + 1, 512], F32, name=f"o_t{ab}")
                            for ab in "AB"]

                    def emit_scores(kt):
                        # packed pair: head A on PE row-group 0-1, head B on
                        # 2-3; both halves land in one 2-bank psum tile so a
                        # single wide exp covers them
                        diag = (kt // 4 == qc)
                        co = 128 * (kt % 4) if diag else 0
                        n = 512 - co
                        ktsl = bass.ts(kt, 128)
                        qsl = bass.ds(qc * 512 + co, n)
                        sc_t = s_ps.tile([128, 1024], F32, name="sc_t")
                        for i, (ro, tp) in enumerate(((0, (0, 0)), (64, (64, 0)))):
                            nc.tensor.matmul(
                                sc_t[:, i * 512:i * 512 + n],
                                qkT[ro:ro + 64, 4 + hp, ktsl],
                                qkT[ro:ro + 64, hp, qsl],
                                start=True, stop=True, tile_position=tp)
                        e_t = exp_pool.tile([128, 1024], MM_DT, name="e_t")
                        if co == 0:
                            nc.scalar.activation(e_t, sc_t, AF.Exp,
                                                 scale=float(1.0 / np.sqrt(DK)))
                        else:
                            for i in range(2):
                                nc.scalar.activation(
                                    e_t[:, i * 512:i * 512 + n],
                                    sc_t[:, i * 512:i * 512 + n], AF.Exp,
                                    scale=float(1.0 / np.sqrt(DK)))
                        if diag:
                            for i in range(2):
                                nc.gpsimd.affine_select(
                                    out=e_t[:, i * 512:i * 512 + 128],
                                    in_=e_t[:, i * 512:i * 512 + 128],
                                    pattern=[[1, 128]], base=0, channel_multiplier=-1,
                                    compare_op=mybir.AluOpType.is_ge, fill=0.0)
                        if dbg is not None and hp == 0:
                            for i in range(2):
                                nc.sync.dma_start(
                                    out=dbg["e_dump"][kt, qc, i, :, 0:n],
                                    in_=e_t.bitcast(F32)[:, i * 512:i * 512 + n])
                        return e_t

                    def emit_av(kt, e_t):
                        diag = (kt // 4 == qc)
                        co = 128 * (kt % 4) if diag else 0
                        n = 512 - co
                        for i, h in enumerate((hA, hB)):
                            nc.tensor.matmul(
                                o_ts[i][:, co:512],
                                vp[:, kt, h * (DK + 1):(h + 1) * (DK + 1)],
                                e_t[:, i * 512:i * 512 + n],
                                start=(kt == 0), stop=(kt == n_kt - 1))

                    # software pipeline: scores(kt+1) is emitted before av(kt)
                    # so the packed score pair stays adjacent in the PE queue
                    prev_e = emit_scores(0)
                    for kt in range(1, n_kt):
                        e_t = emit_scores(kt)
                        emit_av(kt - 1, prev_e)
                        prev_e = e_t
                    emit_av(n_kt - 1, prev_e)
                    for i in range(2):
                        ob = norm_pool.tile([DK + 1, 512], F32, name="ob")
                        nc.vector.tensor_copy(ob, o_ts[i])
                        if dbg is not None and hp == 0:
                            nc.sync.dma_start(out=dbg["o_dump"][qc, i], in_=ob)
                        dsb = norm_pool.tile([1, 512], F32, name="dsb")
                        nc.vector.tensor_copy(dsb, ob[DK:DK + 1, :])
                        recip = norm_pool.tile([1, 512], F32, name="recip")
                        nc.vector.reciprocal_approx_fast(recip, dsb)
                        rb = norm_pool.tile([DK, 512], F32, name="rb")
                        nc.gpsimd.partition_broadcast(rb, recip)
                        hn = norm_pool.tile([DK, 512], F32, name="hn")
                        nc.vector.tensor_mul(hn, ob[0:DK, :], rb)
                        if dbg is not None and hp == 0:
                            nc.sync.dma_start(out=dbg["recip_dump"][qc, i].unsqueeze(0), in_=recip)
                            nc.sync.dma_start(out=dbg["rb_dump"][qc, i], in_=rb)
                            nc.sync.dma_start(out=dbg["hn_dump"][qc, i], in_=hn)
                        nc.sync.dma_start(
                            out=heads_t[i * DK:(i + 1) * DK, hp, bass.ts(qc, 512)],
                            in_=hn.bitcast(MM_DT))
            # last query chunk's o_proj
            for st4 in range(4):
                for nb in range(D // 512):
                    emit_oproj_chain(3, st4, nb)

        if dbg is not None:
            nc.sync.dma_start(out=dbg["heads_dump"], in_=heads_t.bitcast(F32))


def _host_inv_freq():
    import jax
    import jax.numpy as jnp
    with jax.default_device(jax.devices("cpu")[0]):
        v = 1.0 / (THETA ** (jnp.arange(HALF, dtype=jnp.float32) * 2.0 / DK))
        return np.asarray(jax.device_get(v)).astype(np.float32)


_program_cache = None


def _get_program():
    global _program_cache
    if _program_cache is None:
        _program_cache = _build_program()
    return _program_cache


# dk permutation: evens then odds within each head's 64 dims
_PERM64 = np.concatenate([np.arange(0, DK, 2), np.arange(1, DK, 2)])


def _make_in_maps(x, Wq, Wk, Wv, Wo, pos_np):
    invf_np = _host_inv_freq()
    in_maps = []
    for c in range(N_CORES):
        b, hg = c // 2, c % 2
        rows = hg * DPC + np.concatenate(
            [h * DK + _PERM64 for h in range(HPC)])
        in_maps.append({
            "xT": np.ascontiguousarray(x[b].T),
            "wqT": np.ascontiguousarray(Wq[rows, :].T),
            "wkT": np.ascontiguousarray(Wk[rows, :].T),
            "wvT": np.ascontiguousarray(Wv[hg * DPC:(hg + 1) * DPC, :].T),
            "woT": np.ascontiguousarray(Wo[:, hg * DPC:(hg + 1) * DPC].T),
            "pos": pos_np,
            "invf": invf_np,
        })
    return in_maps


def kernel(x, Wq, Wk, Wv, Wo, token_positions):
    x = np.asarray(x, dtype=np.float32)
    Wq = np.asarray(Wq, dtype=np.float32)
    Wk = np.asarray(Wk, dtype=np.float32)
    Wv = np.asarray(Wv, dtype=np.float32)
    Wo = np.asarray(Wo, dtype=np.float32)
    pos_np = np.ascontiguousarray(np.asarray(token_positions, dtype=np.int32))

    nc = _get_program()
    in_maps = _make_in_maps(x, Wq, Wk, Wv, Wo, pos_np)
    res = run_bass_kernel_spmd(nc, in_maps, list(range(N_CORES)))
    out = np.empty((B, S, D), dtype=np.float32)
    for b in range(B):
        out[b] = res.results[2 * b]["y"] + res.results[2 * b + 1]["y"]
    return out
